# revision 18
# baseline (speedup 1.0000x reference)
"""Trainium2 Bass kernel for a 12-head dense attention block (BEiT-style
windowed attention with relative-position bias), batch-parallel over 8
NeuronCores.

Shapes (hardcoded): x [64, 197, 768], qkv_w [2304, 768], proj_w [768, 768],
proj_b [768], rel_table [732, 12], rel_index [197, 197] int32.

Sharding: data-parallel over batch — each of the 8 cores handles 8 batch
elements end-to-end; no collectives. Host pre-transposes x and the weights
so the device kernel needs no on-chip transposes:

  phase 1: qkT[2C, M] = wqkvT.T-style matmul producing q,k TRANSPOSED
           ([feature, token]) + v in natural layout ([token, feature]),
           f32r matmuls (full-rate fp32 path).
  phase 2: per (batch, head): scoresT[nk, nq] = kT.T @ qT, exp on the
           scalar engine, relative-position bias applied multiplicatively
           (exp(bias) precomputed), softmax denominator via gpsimd
           partition_all_reduce, attention output accumulated TRANSPOSED
           (outT[d, nq] = v.T-free matmul) and normalized by a DVE multiply.
  phase 3: y = attn_outT.T @ projT (bf16) + broadcast bias add,
           DMA out in natural layout.
"""

import sys

if "/opt/trn_rl_repo" not in sys.path:
    sys.path.insert(0, "/opt/trn_rl_repo")

import numpy as np
import ml_dtypes

import concourse.bass as bass  # noqa: F401  (registers rust bindings)
import concourse.tile as tile
from concourse import bacc, bass_isa, mybir
from concourse.bass_utils import run_bass_kernel_spmd

N_CORES = 8
B, N, C, H, D = 64, 197, 768, 12, 64
BL = B // N_CORES            # 8 batch elements per core
M = BL * N                   # 1576 tokens per core
SCALE = D ** -0.5
NK0 = 128
NK1 = N - NK0                # 69
KC = C // 128                # 6 contraction chunks
MT = 4                       # m-tiles in phase 1 (qk part)
MTS = M // MT                # 394
NT_QK = (2 * C) // 128       # 12 output-feature tiles for q,k
MT3 = (M + 127) // 128       # 13 m-tiles in phase 3

F32 = mybir.dt.float32
BF16 = mybir.dt.bfloat16
F32R = mybir.dt.float32r

# self-inverse head<->slot permutation within each 4-head group: consecutive
# score matmuls alternate array row-strips (head parity) and run concurrently,
# so they must target different PSUM banks -> interleave slots (0,2,1,3)
SIG4 = (0, 2, 1, 3)


def sig(h):
    return (h // 4) * 4 + SIG4[h % 4]


_COMPILED = {}


def _build_nc():
    nc = bacc.Bacc(
        "TRN2", target_bir_lowering=False, debug=False, num_devices=N_CORES
    )
    xT = nc.declare_dram_parameter("xT", [C, M], F32R, isOutput=False)
    wqkvT = nc.declare_dram_parameter("wqkvT", [C, 3 * C], F32R, isOutput=False)
    wprojT = nc.declare_dram_parameter("wprojT", [C, C], BF16, isOutput=False)
    projb = nc.declare_dram_parameter("projb", [1, C], F32, isOutput=False)
    biasT = nc.declare_dram_parameter("biasT", [2, 128, H, N], BF16, isOutput=False)
    out_d = nc.declare_dram_parameter("out", [M, C], F32, isOutput=True)

    with tile.TileContext(nc) as tc:
        _body(nc, tc, xT, wqkvT, wprojT, projb, biasT, out_d)
    nc.compile()
    return nc


def _body(nc, tc, xT, wqkvT, wprojT, projb, biasT, out_d):
    exp = mybir.ActivationFunctionType.Exp

    consts = tc.alloc_tile_pool(name="consts", bufs=1)
    projb_sb = consts.tile([1, C], F32)
    nc.sync.dma_start(out=projb_sb[:, :], in_=projb[:, :])
    projb_bc = consts.tile([128, C], F32)
    nc.gpsimd.partition_broadcast(projb_bc[:, :], projb_sb[:, :], channels=128)
    bias_sb = [consts.tile([128, H, N], BF16, tag=f"bias{t}", name=f"bias{t}") for t in range(2)]
    for t in range(2):
        nc.sync.dma_start(out=bias_sb[t][:, :, :], in_=biasT[t, :, :, :])

    # ---- outputs of phase 1 (persist into phase 2) ----
    qk_pool = tc.alloc_tile_pool(name="qk", bufs=1)
    qkT = [qk_pool.tile([128, M], BF16, tag=f"qk{t}", name=f"qk{t}") for t in range(NT_QK)]
    v_pool = tc.alloc_tile_pool(name="v", bufs=1)
    v_sb = [
        [v_pool.tile([128, C], BF16, tag=f"v{b}_{pt}", name=f"v{b}_{pt}") for pt in range(2)]
        for b in range(BL)
    ]

    ps_mm = tc.alloc_tile_pool(name="psmm", bufs=2, space="PSUM")

    # ---- phase 1: qkT (transposed q,k) and v (natural) ----
    with tc.tile_pool(name="xt", bufs=1) as xt_pool, tc.tile_pool(
        name="wq", bufs=1
    ) as wq_pool:
        xt = [xt_pool.tile([128, M], F32R, tag=f"xt{k}", name=f"xt{k}") for k in range(KC)]
        wq = [wq_pool.tile([128, 3 * C], F32R, tag=f"wq{k}", name=f"wq{k}") for k in range(KC)]
        for k in range(KC):
            nc.sync.dma_start(out=xt[k][:, :], in_=xT[k * 128 : (k + 1) * 128, :])
            nc.sync.dma_start(
                out=wq[k][:, :], in_=wqkvT[k * 128 : (k + 1) * 128, :]
            )

        for mt in range(MT):
            ms = slice(mt * MTS, (mt + 1) * MTS)
            for nt in range(NT_QK):
                ps = ps_mm.tile([128, MTS], F32, tag="ps1")
                for k in range(KC):
                    nc.tensor.matmul(
                        ps[:, :],
                        wq[k][:, nt * 128 : (nt + 1) * 128],
                        xt[k][:, ms],
                        start=(k == 0),
                        stop=(k == KC - 1),
                    )
                nc.any.tensor_copy(qkT[nt][:, ms], ps[:, :])
            for b in (2 * mt, 2 * mt + 1):
                for pt in range(2):
                    psz = NK0 if pt == 0 else NK1
                    mofs = b * N + pt * 128
                    for nt2 in range(2):
                        ps = ps_mm.tile([128, 384], F32, tag="ps1")
                        for k in range(KC):
                            nc.tensor.matmul(
                                ps[:psz, :],
                                xt[k][:, mofs : mofs + psz],
                                wq[k][
                                    :, 2 * C + nt2 * 384 : 2 * C + (nt2 + 1) * 384
                                ],
                                start=(k == 0),
                                stop=(k == KC - 1),
                            )
                        nc.any.tensor_copy(
                            v_sb[b][pt][:psz, nt2 * 384 : (nt2 + 1) * 384],
                            ps[:psz, :],
                        )

    # ---- phase 3 weights: load early into space freed by xt/wq ----
    wp_pool = tc.alloc_tile_pool(name="wp", bufs=1)
    wp = [wp_pool.tile([128, C], BF16, tag=f"wp{k}", name=f"wp{k}") for k in range(KC)]
    for k in range(KC):
        nc.sync.dma_start(out=wp[k][:, :], in_=wprojT[k * 128 : (k + 1) * 128, :])

    ao_pool = tc.alloc_tile_pool(name="ao", bufs=1)
    aoT = [ao_pool.tile([128, M], BF16, tag=f"ao{t}", name=f"ao{t}") for t in range(KC)]

    # ---- phase 2: attention per batch element ----
    et_pool = tc.alloc_tile_pool(name="et", bufs=2)
    raw_pool = tc.alloc_tile_pool(name="raw", bufs=3)
    ar_pool = tc.alloc_tile_pool(name="ar", bufs=2)
    ps_sc = tc.alloc_tile_pool(name="pssc", bufs=2, space="PSUM")
    ps_po = tc.alloc_tile_pool(name="pspo", bufs=2, space="PSUM")

    for b in range(BL):
        et = et_pool.tile([128, H, 2, N], BF16, tag="et")
        for kt in range(2):
            nk = NK0 if kt == 0 else NK1
            kofs = b * N + kt * 128
            for hg in range(3):
                ps = ps_sc.tile([128, 4, 256], F32, tag="pssc")
                for j in range(4):
                    h = hg * 4 + j
                    off = (h % 2) * 64
                    # scoresT[nk, nq] = kT.T @ qT  (scale folded into Wq);
                    # psum slice SIG4[j] so concurrent row-packed MMs use
                    # different banks
                    nc.tensor.matmul(
                        ps[:nk, SIG4[j], 0:N],
                        qkT[6 + h // 2][off : off + 64, kofs : kofs + nk],
                        qkT[h // 2][off : off + 64, b * N : b * N + N],
                        start=True,
                        stop=True,
                    )
                raw = raw_pool.tile([128, 4, N], BF16, tag="raw")
                nc.scalar.activation(raw[:nk, :, :], ps[:nk, :, 0:N], exp)
                # multiplicative relative-position bias: et = exp(z)*exp(bias)
                nc.gpsimd.tensor_mul(
                    et[:nk, hg * 4 : (hg + 1) * 4, kt, :],
                    raw[:nk, :, :],
                    bias_sb[kt][:nk, hg * 4 : (hg + 1) * 4, :],
                )
        # per-(head, nq) softmax denominator: sum the two nk tiles (tile 1
        # only has 69 valid partitions), all-reduce across partitions
        # (result broadcast to every partition), then reciprocal in place
        su = ar_pool.tile([128, H, N], BF16, tag="su")
        nc.vector.tensor_copy(su[:, :, :], et[:, :, 0, :])
        nc.vector.tensor_add(su[:NK1, :, :], su[:NK1, :, :], et[:NK1, :, 1, :])
        ar = ar_pool.tile([128, H, N], F32, tag="ar")
        nc.gpsimd.partition_all_reduce(
            ar[:, :, :],
            su[:, :, :],
            channels=128,
            reduce_op=bass_isa.ReduceOp.add,
        )
        nc.vector.reciprocal_approx_fast(out=ar[:, :, :], in_=ar[:, :, :])

        for grp in range(3):
            po = ps_po.tile([128, 2, 256], F32, tag="pspo")
            for j in range(4):
                h = grp * 4 + j
                base = (j % 2) * 64
                sl = j // 2
                for kt in range(2):
                    nk = NK0 if kt == 0 else NK1
                    nc.tensor.matmul(
                        po[base : base + 64, sl, 0:N],
                        v_sb[b][kt][:nk, h * 64 : (h + 1) * 64],
                        et[:nk, sig(h), kt, :],
                        start=(kt == 0),
                        stop=(kt == 1),
                        tile_position=(0, base),
                    )
            for j in range(4):
                h = grp * 4 + j
                base = (j % 2) * 64
                sl = j // 2
                nc.vector.tensor_tensor(
                    aoT[h // 2][base : base + 64, b * N : b * N + N],
                    po[base : base + 64, sl, 0:N],
                    ar[base : base + 64, sig(h), :],
                    mybir.AluOpType.mult,
                )

    # ---- phase 3: y = attn_outT.T @ projT + proj_b ----
    with tc.tile_pool(name="ostg", bufs=3) as ostg_pool:
        for mt in range(MT3):
            msz = 128 if mt < MT3 - 1 else M - 128 * (MT3 - 1)
            stg = ostg_pool.tile([128, C], F32, tag="stg")
            for nt2 in range(2):
                ns = slice(nt2 * 384, (nt2 + 1) * 384)
                ps = ps_mm.tile([128, 384], F32, tag="ps1")
                for c in range(KC):
                    nc.tensor.matmul(
                        ps[:msz, :],
                        aoT[c][:, mt * 128 : mt * 128 + msz],
                        wp[c][:, ns],
                        start=(c == 0),
                        stop=(c == KC - 1),
                    )
                nc.any.tensor_add(stg[:msz, ns], ps[:msz, :], projb_bc[:msz, ns])
            nc.sync.dma_start(
                out=out_d[mt * 128 : mt * 128 + msz, :], in_=stg[:msz, :]
            )

    for pool in (
        ps_po,
        ps_sc,
        ar_pool,
        raw_pool,
        et_pool,
        ao_pool,
        wp_pool,
        ps_mm,
        v_pool,
        qk_pool,
        consts,
    ):
        pool.release()


def _get_compiled():
    if "nc" not in _COMPILED:
        _COMPILED["nc"] = _build_nc()
    return _COMPILED["nc"]


def _prep_host(inputs):
    qkv_w = np.asarray(inputs["qkv_w"], dtype=np.float32)
    proj_w = np.asarray(inputs["proj_w"], dtype=np.float32)
    proj_b = np.asarray(inputs["proj_b"], dtype=np.float32)
    rel_table = np.asarray(inputs["rel_table"], dtype=np.float32)
    rel_index = np.asarray(inputs["rel_index"]).astype(np.int64)

    w = qkv_w.copy()
    w[:C] *= SCALE  # fold the attention scale into Wq
    wqkvT = np.ascontiguousarray(w.T)
    wprojT = np.ascontiguousarray(proj_w.T).astype(ml_dtypes.bfloat16)
    projb2 = np.ascontiguousarray(proj_b.reshape(1, C))

    bias_full = rel_table[rel_index]          # [nq, nk, H]
    biasT = np.exp(bias_full.transpose(2, 1, 0))  # [H, nk, nq], exp for the
    # multiplicative-bias trick: exp(z + b) = exp(z) * exp(b)
    perm = [(t // 4) * 4 + (0, 2, 1, 3)[t % 4] for t in range(H)]
    biasT = biasT[perm]
    pad = np.zeros((H, 2 * 128, N), np.float32)
    pad[:, :N, :] = biasT
    bias_dev = np.ascontiguousarray(
        pad.reshape(H, 2, 128, N).transpose(1, 2, 0, 3)
    ).astype(ml_dtypes.bfloat16)
    return wqkvT, wprojT, projb2, bias_dev


def kernel(**inputs):
    x = np.asarray(inputs["x"], dtype=np.float32)
    wqkvT, wprojT, projb2, bias_dev = _prep_host(inputs)

    nc = _get_compiled()
    in_maps = []
    for i in range(N_CORES):
        shard = x[i * BL : (i + 1) * BL].reshape(M, C)
        in_maps.append(
            {
                "xT": np.ascontiguousarray(shard.T),
                "wqkvT": wqkvT,
                "wprojT": wprojT,
                "projb": projb2,
                "biasT": bias_dev,
            }
        )
    res = run_bass_kernel_spmd(nc, in_maps, core_ids=list(range(N_CORES)))
    out = np.empty((B, N, C), dtype=np.float32)
    for i in range(N_CORES):
        out[i * BL : (i + 1) * BL] = res.results[i]["out"].reshape(BL, N, C)
    return out


def run_traced(**inputs):
    """Like kernel() but with NTFF tracing; returns (out, BassKernelResults)."""
    x = np.asarray(inputs["x"], dtype=np.float32)
    wqkvT, wprojT, projb2, bias_dev = _prep_host(inputs)
    nc = _get_compiled()
    in_maps = []
    for i in range(N_CORES):
        shard = x[i * BL : (i + 1) * BL].reshape(M, C)
        in_maps.append(
            {
                "xT": np.ascontiguousarray(shard.T),
                "wqkvT": wqkvT,
                "wprojT": wprojT,
                "projb": projb2,
                "biasT": bias_dev,
            }
        )
    res = run_bass_kernel_spmd(
        nc, in_maps, core_ids=list(range(N_CORES)), trace=True
    )
    out = np.empty((B, N, C), dtype=np.float32)
    for i in range(N_CORES):
        out[i * BL : (i + 1) * BL] = res.results[i]["out"].reshape(BL, N, C)
    return out, res


# revision 19
# speedup vs baseline: 1.3255x; 1.3255x over previous
"""Trainium2 Bass kernel for a 12-head dense attention block (BEiT-style
windowed attention with relative-position bias), batch-parallel over 8
NeuronCores.

Shapes (hardcoded): x [64, 197, 768], qkv_w [2304, 768], proj_w [768, 768],
proj_b [768], rel_table [732, 12], rel_index [197, 197] int32.

Sharding: data-parallel over batch — each of the 8 cores handles 8 batch
elements end-to-end; no collectives. Host pre-transposes x and the weights
so the device kernel needs no on-chip transposes:

  phase 1: qkT[2C, M] = wqkvT.T-style matmul producing q,k TRANSPOSED
           ([feature, token]) + v in natural layout ([token, feature]),
           f32r matmuls (full-rate fp32 path).
  phase 2: per (batch, head): scoresT[nk, nq] = kT.T @ qT, exp on the
           scalar engine, relative-position bias applied multiplicatively
           (exp(bias) precomputed), softmax denominator via gpsimd
           partition_all_reduce, attention output accumulated TRANSPOSED
           (outT[d, nq] = v.T-free matmul) and normalized by a DVE multiply.
  phase 3: y = attn_outT.T @ projT (bf16) + broadcast bias add,
           DMA out in natural layout.
"""

import sys

if "/opt/trn_rl_repo" not in sys.path:
    sys.path.insert(0, "/opt/trn_rl_repo")

import numpy as np
import ml_dtypes

import concourse.bass as bass  # noqa: F401  (registers rust bindings)
import concourse.tile as tile
from concourse import bacc, bass_isa, mybir
from concourse.bass_utils import run_bass_kernel_spmd

N_CORES = 8
B, N, C, H, D = 64, 197, 768, 12, 64
BL = B // N_CORES            # 8 batch elements per core
M = BL * N                   # 1576 tokens per core
SCALE = D ** -0.5
NK0 = 128
NK1 = N - NK0                # 69
KC = C // 128                # 6 contraction chunks
MT = 4                       # m-tiles in phase 1 (qk part)
MTS = M // MT                # 394
NT_QK = (2 * C) // 128       # 12 output-feature tiles for q,k
MT3 = (M + 127) // 128       # 13 m-tiles in phase 3

F32 = mybir.dt.float32
BF16 = mybir.dt.bfloat16
F32R = mybir.dt.float32r

# self-inverse head<->slot permutation within each 4-head group: consecutive
# score matmuls alternate array row-strips (head parity) and run concurrently,
# so they must target different PSUM banks -> interleave slots (0,2,1,3)
SIG4 = (0, 2, 1, 3)


def sig(h):
    return (h // 4) * 4 + SIG4[h % 4]


_COMPILED = {}


def _build_nc():
    nc = bacc.Bacc(
        "TRN2", target_bir_lowering=False, debug=False, num_devices=N_CORES
    )
    xT = nc.declare_dram_parameter("xT", [C, M], F32R, isOutput=False)
    wqkvT = nc.declare_dram_parameter("wqkvT", [C, 3 * C], F32R, isOutput=False)
    wprojT = nc.declare_dram_parameter("wprojT", [C, C], BF16, isOutput=False)
    projb = nc.declare_dram_parameter("projb", [1, C], F32, isOutput=False)
    biasT = nc.declare_dram_parameter("biasT", [2, 128, H, N], BF16, isOutput=False)
    out_d = nc.declare_dram_parameter("out", [M, C], F32, isOutput=True)

    with tile.TileContext(nc) as tc:
        _body(nc, tc, xT, wqkvT, wprojT, projb, biasT, out_d)
    nc.compile()
    return nc


def _body(nc, tc, xT, wqkvT, wprojT, projb, biasT, out_d):
    exp = mybir.ActivationFunctionType.Exp

    consts = tc.alloc_tile_pool(name="consts", bufs=1)
    projb_sb = consts.tile([1, C], F32)
    nc.sync.dma_start(out=projb_sb[:, :], in_=projb[:, :])
    projb_bc = consts.tile([128, C], F32)
    nc.gpsimd.partition_broadcast(projb_bc[:, :], projb_sb[:, :], channels=128)
    bias_sb = [consts.tile([128, H, N], BF16, tag=f"bias{t}", name=f"bias{t}") for t in range(2)]
    for t in range(2):
        nc.sync.dma_start(out=bias_sb[t][:, :, :], in_=biasT[t, :, :, :])

    # ---- outputs of phase 1 (persist into phase 2) ----
    qk_pool = tc.alloc_tile_pool(name="qk", bufs=1)
    qkT = [qk_pool.tile([128, M], BF16, tag=f"qk{t}", name=f"qk{t}") for t in range(NT_QK)]
    v_pool = tc.alloc_tile_pool(name="v", bufs=1)
    v_sb = [
        [v_pool.tile([128, C], BF16, tag=f"v{b}_{pt}", name=f"v{b}_{pt}") for pt in range(2)]
        for b in range(BL)
    ]

    ps_mm = tc.alloc_tile_pool(name="psmm", bufs=2, space="PSUM")

    # ---- phase 1: qkT (transposed q,k) and v (natural) ----
    with tc.tile_pool(name="xt", bufs=1) as xt_pool, tc.tile_pool(
        name="wq", bufs=1
    ) as wq_pool:
        xt = [xt_pool.tile([128, M], F32R, tag=f"xt{k}", name=f"xt{k}") for k in range(KC)]
        wq = [wq_pool.tile([128, 3 * C], F32R, tag=f"wq{k}", name=f"wq{k}") for k in range(KC)]
        for k in range(KC):
            nc.sync.dma_start(out=xt[k][:, :], in_=xT[k * 128 : (k + 1) * 128, :])
            nc.sync.dma_start(
                out=wq[k][:, :], in_=wqkvT[k * 128 : (k + 1) * 128, :]
            )

        for mt in range(MT):
            ms = slice(mt * MTS, (mt + 1) * MTS)
            for nt in range(NT_QK):
                ps = ps_mm.tile([128, MTS], F32, tag="ps1")
                for k in range(KC):
                    nc.tensor.matmul(
                        ps[:, :],
                        wq[k][:, nt * 128 : (nt + 1) * 128],
                        xt[k][:, ms],
                        start=(k == 0),
                        stop=(k == KC - 1),
                    )
                nc.any.tensor_copy(qkT[nt][:, ms], ps[:, :])
            for b in (2 * mt, 2 * mt + 1):
                for pt in range(2):
                    psz = NK0 if pt == 0 else NK1
                    mofs = b * N + pt * 128
                    for nt2 in range(2):
                        ps = ps_mm.tile([128, 384], F32, tag="ps1")
                        for k in range(KC):
                            nc.tensor.matmul(
                                ps[:psz, :],
                                xt[k][:, mofs : mofs + psz],
                                wq[k][
                                    :, 2 * C + nt2 * 384 : 2 * C + (nt2 + 1) * 384
                                ],
                                start=(k == 0),
                                stop=(k == KC - 1),
                            )
                        nc.any.tensor_copy(
                            v_sb[b][pt][:psz, nt2 * 384 : (nt2 + 1) * 384],
                            ps[:psz, :],
                        )

    # ---- phase 3 weights: load early into space freed by xt/wq ----
    wp_pool = tc.alloc_tile_pool(name="wp", bufs=1)
    wp = [wp_pool.tile([128, C], BF16, tag=f"wp{k}", name=f"wp{k}") for k in range(KC)]
    for k in range(KC):
        nc.sync.dma_start(out=wp[k][:, :], in_=wprojT[k * 128 : (k + 1) * 128, :])

    ao_pool = tc.alloc_tile_pool(name="ao", bufs=1)
    aoT = [ao_pool.tile([128, M], BF16, tag=f"ao{t}", name=f"ao{t}") for t in range(KC)]

    # ---- phase 2: attention per batch element ----
    et_pool = tc.alloc_tile_pool(name="et", bufs=2)
    raw_pool = tc.alloc_tile_pool(name="raw", bufs=3)
    ar_pool = tc.alloc_tile_pool(name="ar", bufs=2)
    ps_sc = tc.alloc_tile_pool(name="pssc", bufs=2, space="PSUM")
    ps_po = tc.alloc_tile_pool(name="pspo", bufs=2, space="PSUM")

    for b in range(BL):
        et = et_pool.tile([128, H, 2, N], BF16, tag="et")
        for kt in range(2):
            nk = NK0 if kt == 0 else NK1
            kofs = b * N + kt * 128
            for hg in range(3):
                ps = ps_sc.tile([128, 4, 256], F32, tag="pssc")
                for j in range(4):
                    h = hg * 4 + j
                    off = (h % 2) * 64
                    # scoresT[nk, nq] = kT.T @ qT  (scale folded into Wq);
                    # psum slice SIG4[j] so concurrent row-packed MMs use
                    # different banks
                    nc.tensor.matmul(
                        ps[:nk, SIG4[j], 0:N],
                        qkT[6 + h // 2][off : off + 64, kofs : kofs + nk],
                        qkT[h // 2][off : off + 64, b * N : b * N + N],
                        start=True,
                        stop=True,
                    )
                raw = raw_pool.tile([128, 4, N], BF16, tag="raw")
                nc.scalar.activation(raw[:nk, :, :], ps[:nk, :, 0:N], exp)
                # multiplicative relative-position bias: et = exp(z)*exp(bias)
                nc.vector.tensor_mul(
                    et[:nk, hg * 4 : (hg + 1) * 4, kt, :],
                    raw[:nk, :, :],
                    bias_sb[kt][:nk, hg * 4 : (hg + 1) * 4, :],
                )
        # per-(head, nq) softmax denominator: sum the two nk tiles (tile 1
        # only has 69 valid partitions), all-reduce across partitions
        # (result broadcast to every partition), then reciprocal in place
        su = ar_pool.tile([128, H, N], BF16, tag="su")
        nc.vector.tensor_copy(su[:, :, :], et[:, :, 0, :])
        nc.vector.tensor_add(su[:NK1, :, :], su[:NK1, :, :], et[:NK1, :, 1, :])
        ar = ar_pool.tile([128, H, N], F32, tag="ar")
        nc.gpsimd.partition_all_reduce(
            ar[:, :, :],
            su[:, :, :],
            channels=128,
            reduce_op=bass_isa.ReduceOp.add,
        )
        nc.vector.reciprocal_approx_fast(out=ar[:, :, :], in_=ar[:, :, :])

        for grp in range(3):
            po = ps_po.tile([128, 2, 256], F32, tag="pspo")
            for j in range(4):
                h = grp * 4 + j
                base = (j % 2) * 64
                sl = j // 2
                for kt in range(2):
                    nk = NK0 if kt == 0 else NK1
                    nc.tensor.matmul(
                        po[base : base + 64, sl, 0:N],
                        v_sb[b][kt][:nk, h * 64 : (h + 1) * 64],
                        et[:nk, sig(h), kt, :],
                        start=(kt == 0),
                        stop=(kt == 1),
                        tile_position=(0, base),
                    )
            for j in range(4):
                h = grp * 4 + j
                base = (j % 2) * 64
                sl = j // 2
                nc.vector.tensor_tensor(
                    aoT[h // 2][base : base + 64, b * N : b * N + N],
                    po[base : base + 64, sl, 0:N],
                    ar[base : base + 64, sig(h), :],
                    mybir.AluOpType.mult,
                )

    # ---- phase 3: y = attn_outT.T @ projT + proj_b ----
    with tc.tile_pool(name="ostg", bufs=3) as ostg_pool:
        for mt in range(MT3):
            msz = 128 if mt < MT3 - 1 else M - 128 * (MT3 - 1)
            stg = ostg_pool.tile([128, C], F32, tag="stg")
            for nt2 in range(2):
                ns = slice(nt2 * 384, (nt2 + 1) * 384)
                ps = ps_mm.tile([128, 384], F32, tag="ps1")
                for c in range(KC):
                    nc.tensor.matmul(
                        ps[:msz, :],
                        aoT[c][:, mt * 128 : mt * 128 + msz],
                        wp[c][:, ns],
                        start=(c == 0),
                        stop=(c == KC - 1),
                    )
                nc.any.tensor_add(stg[:msz, ns], ps[:msz, :], projb_bc[:msz, ns])
            nc.sync.dma_start(
                out=out_d[mt * 128 : mt * 128 + msz, :], in_=stg[:msz, :]
            )

    for pool in (
        ps_po,
        ps_sc,
        ar_pool,
        raw_pool,
        et_pool,
        ao_pool,
        wp_pool,
        ps_mm,
        v_pool,
        qk_pool,
        consts,
    ):
        pool.release()


def _get_compiled():
    if "nc" not in _COMPILED:
        _COMPILED["nc"] = _build_nc()
    return _COMPILED["nc"]


def _prep_host(inputs):
    qkv_w = np.asarray(inputs["qkv_w"], dtype=np.float32)
    proj_w = np.asarray(inputs["proj_w"], dtype=np.float32)
    proj_b = np.asarray(inputs["proj_b"], dtype=np.float32)
    rel_table = np.asarray(inputs["rel_table"], dtype=np.float32)
    rel_index = np.asarray(inputs["rel_index"]).astype(np.int64)

    w = qkv_w.copy()
    w[:C] *= SCALE  # fold the attention scale into Wq
    wqkvT = np.ascontiguousarray(w.T)
    wprojT = np.ascontiguousarray(proj_w.T).astype(ml_dtypes.bfloat16)
    projb2 = np.ascontiguousarray(proj_b.reshape(1, C))

    bias_full = rel_table[rel_index]          # [nq, nk, H]
    biasT = np.exp(bias_full.transpose(2, 1, 0))  # [H, nk, nq], exp for the
    # multiplicative-bias trick: exp(z + b) = exp(z) * exp(b)
    perm = [(t // 4) * 4 + (0, 2, 1, 3)[t % 4] for t in range(H)]
    biasT = biasT[perm]
    pad = np.zeros((H, 2 * 128, N), np.float32)
    pad[:, :N, :] = biasT
    bias_dev = np.ascontiguousarray(
        pad.reshape(H, 2, 128, N).transpose(1, 2, 0, 3)
    ).astype(ml_dtypes.bfloat16)
    return wqkvT, wprojT, projb2, bias_dev


def kernel(**inputs):
    x = np.asarray(inputs["x"], dtype=np.float32)
    wqkvT, wprojT, projb2, bias_dev = _prep_host(inputs)

    nc = _get_compiled()
    in_maps = []
    for i in range(N_CORES):
        shard = x[i * BL : (i + 1) * BL].reshape(M, C)
        in_maps.append(
            {
                "xT": np.ascontiguousarray(shard.T),
                "wqkvT": wqkvT,
                "wprojT": wprojT,
                "projb": projb2,
                "biasT": bias_dev,
            }
        )
    res = run_bass_kernel_spmd(nc, in_maps, core_ids=list(range(N_CORES)))
    out = np.empty((B, N, C), dtype=np.float32)
    for i in range(N_CORES):
        out[i * BL : (i + 1) * BL] = res.results[i]["out"].reshape(BL, N, C)
    return out


def run_traced(**inputs):
    """Like kernel() but with NTFF tracing; returns (out, BassKernelResults)."""
    x = np.asarray(inputs["x"], dtype=np.float32)
    wqkvT, wprojT, projb2, bias_dev = _prep_host(inputs)
    nc = _get_compiled()
    in_maps = []
    for i in range(N_CORES):
        shard = x[i * BL : (i + 1) * BL].reshape(M, C)
        in_maps.append(
            {
                "xT": np.ascontiguousarray(shard.T),
                "wqkvT": wqkvT,
                "wprojT": wprojT,
                "projb": projb2,
                "biasT": bias_dev,
            }
        )
    res = run_bass_kernel_spmd(
        nc, in_maps, core_ids=list(range(N_CORES)), trace=True
    )
    out = np.empty((B, N, C), dtype=np.float32)
    for i in range(N_CORES):
        out[i * BL : (i + 1) * BL] = res.results[i]["out"].reshape(BL, N, C)
    return out, res


# revision 22
# speedup vs baseline: 1.7035x; 1.2852x over previous
"""Trainium2 Bass kernel for a 12-head dense attention block (BEiT-style
windowed attention with relative-position bias), batch-parallel over 8
NeuronCores.

Shapes (hardcoded): x [64, 197, 768], qkv_w [2304, 768], proj_w [768, 768],
proj_b [768], rel_table [732, 12], rel_index [197, 197] int32.

Sharding: data-parallel over batch — each of the 8 cores handles 8 batch
elements end-to-end; no collectives. Host pre-transposes x and the weights
so the device kernel needs no on-chip transposes:

  phase 1: qkT[2C, M] = wqkvT.T-style matmul producing q,k TRANSPOSED
           ([feature, token]) + v in natural layout ([token, feature]),
           f32r matmuls (full-rate fp32 path).
  phase 2: per (batch, head): scoresT[nk, nq] = kT.T @ qT, exp on the
           scalar engine, relative-position bias applied multiplicatively
           (exp(bias) precomputed), softmax denominator via gpsimd
           partition_all_reduce, attention output accumulated TRANSPOSED
           (outT[d, nq] = v.T-free matmul) and normalized by a DVE multiply.
  phase 3: y = attn_outT.T @ projT (bf16) + broadcast bias add,
           DMA out in natural layout.
"""

import sys

if "/opt/trn_rl_repo" not in sys.path:
    sys.path.insert(0, "/opt/trn_rl_repo")

import numpy as np
import ml_dtypes

import concourse.bass as bass  # noqa: F401  (registers rust bindings)
import concourse.tile as tile
from concourse import bacc, bass_isa, mybir
from concourse.bass_utils import run_bass_kernel_spmd

N_CORES = 8
B, N, C, H, D = 64, 197, 768, 12, 64
BL = B // N_CORES            # 8 batch elements per core
M = BL * N                   # 1576 tokens per core
SCALE = D ** -0.5
NK0 = 128
NK1 = N - NK0                # 69
KC = C // 128                # 6 contraction chunks
MT = 4                       # m-tiles in phase 1 (qk part)
MTS = M // MT                # 394
NT_QK = (2 * C) // 128       # 12 output-feature tiles for q,k
MT3 = (M + 127) // 128       # 13 m-tiles in phase 3

F32 = mybir.dt.float32
BF16 = mybir.dt.bfloat16
F32R = mybir.dt.float32r

# self-inverse head<->slot permutation within each 4-head group: consecutive
# score matmuls alternate array row-strips (head parity) and run concurrently,
# so they must target different PSUM banks -> interleave slots (0,2,1,3)
SIG4 = (0, 2, 1, 3)


def sig(h):
    return (h // 4) * 4 + SIG4[h % 4]


_COMPILED = {}


def _build_nc():
    nc = bacc.Bacc(
        "TRN2", target_bir_lowering=False, debug=False, num_devices=N_CORES
    )
    xT = nc.declare_dram_parameter("xT", [C, M], F32R, isOutput=False)
    wqkvT = nc.declare_dram_parameter("wqkvT", [C, 3 * C], F32R, isOutput=False)
    wprojT = nc.declare_dram_parameter("wprojT", [C, C], BF16, isOutput=False)
    projb = nc.declare_dram_parameter("projb", [1, C], F32, isOutput=False)
    biasT = nc.declare_dram_parameter("biasT", [2, 128, H, N], BF16, isOutput=False)
    out_d = nc.declare_dram_parameter("out", [M, C], F32, isOutput=True)

    with tile.TileContext(nc) as tc:
        _body(nc, tc, xT, wqkvT, wprojT, projb, biasT, out_d)
    nc.compile()
    return nc


def _body(nc, tc, xT, wqkvT, wprojT, projb, biasT, out_d):
    exp = mybir.ActivationFunctionType.Exp

    consts = tc.alloc_tile_pool(name="consts", bufs=1)
    ones128 = consts.tile([128, 128], BF16)
    nc.vector.memset(ones128, 1.0)
    projb_sb = consts.tile([1, C], F32)
    nc.sync.dma_start(out=projb_sb[:, :], in_=projb[:, :])
    projb_bc = consts.tile([128, C], F32)
    _pb = projb[:, :]
    nc.sync.dma_start(
        out=projb_bc[:, :],
        in_=bass.AP(tensor=_pb.tensor, offset=_pb.offset, ap=[[0, 128], [1, C]]),
    )
    bias_sb = [consts.tile([128, H, N], BF16, tag=f"bias{t}", name=f"bias{t}") for t in range(2)]
    for t in range(2):
        nc.sync.dma_start(out=bias_sb[t][:, :, :], in_=biasT[t, :, :, :])

    # ---- outputs of phase 1 (persist into phase 2) ----
    qk_pool = tc.alloc_tile_pool(name="qk", bufs=1)
    qkT = [qk_pool.tile([128, M], BF16, tag=f"qk{t}", name=f"qk{t}") for t in range(NT_QK)]
    v_pool = tc.alloc_tile_pool(name="v", bufs=1)
    v_sb = [
        [v_pool.tile([128, C], BF16, tag=f"v{b}_{pt}", name=f"v{b}_{pt}") for pt in range(2)]
        for b in range(BL)
    ]

    ps_mm = tc.alloc_tile_pool(name="psmm", bufs=2, space="PSUM")

    # ---- phase 1: qkT (transposed q,k) and v (natural) ----
    with tc.tile_pool(name="xt", bufs=1) as xt_pool, tc.tile_pool(
        name="wq", bufs=1
    ) as wq_pool:
        xt = [xt_pool.tile([128, M], F32R, tag=f"xt{k}", name=f"xt{k}") for k in range(KC)]
        wq = [wq_pool.tile([128, 3 * C], F32R, tag=f"wq{k}", name=f"wq{k}") for k in range(KC)]
        for k in range(KC):
            nc.sync.dma_start(out=xt[k][:, :], in_=xT[k * 128 : (k + 1) * 128, :])
            nc.sync.dma_start(
                out=wq[k][:, :], in_=wqkvT[k * 128 : (k + 1) * 128, :]
            )

        for mt in range(MT):
            ms = slice(mt * MTS, (mt + 1) * MTS)
            for nt in range(NT_QK):
                ps = ps_mm.tile([128, MTS], F32, tag="ps1")
                for k in range(KC):
                    nc.tensor.matmul(
                        ps[:, :],
                        wq[k][:, nt * 128 : (nt + 1) * 128],
                        xt[k][:, ms],
                        start=(k == 0),
                        stop=(k == KC - 1),
                    )
                nc.any.tensor_copy(qkT[nt][:, ms], ps[:, :])
            for b in (2 * mt, 2 * mt + 1):
                for pt in range(2):
                    psz = NK0 if pt == 0 else NK1
                    mofs = b * N + pt * 128
                    for nt2 in range(2):
                        ps = ps_mm.tile([128, 384], F32, tag="ps1")
                        for k in range(KC):
                            nc.tensor.matmul(
                                ps[:psz, :],
                                xt[k][:, mofs : mofs + psz],
                                wq[k][
                                    :, 2 * C + nt2 * 384 : 2 * C + (nt2 + 1) * 384
                                ],
                                start=(k == 0),
                                stop=(k == KC - 1),
                            )
                        nc.any.tensor_copy(
                            v_sb[b][pt][:psz, nt2 * 384 : (nt2 + 1) * 384],
                            ps[:psz, :],
                        )

    # ---- phase 3 weights: load early into space freed by xt/wq ----
    wp_pool = tc.alloc_tile_pool(name="wp", bufs=1)
    wp = [wp_pool.tile([128, C], BF16, tag=f"wp{k}", name=f"wp{k}") for k in range(KC)]
    for k in range(KC):
        nc.sync.dma_start(out=wp[k][:, :], in_=wprojT[k * 128 : (k + 1) * 128, :])

    ao_pool = tc.alloc_tile_pool(name="ao", bufs=1)
    aoT = [ao_pool.tile([128, M], BF16, tag=f"ao{t}", name=f"ao{t}") for t in range(KC)]

    # ---- phase 2: attention per batch element ----
    et_pool = tc.alloc_tile_pool(name="et", bufs=2)
    raw_pool = tc.alloc_tile_pool(name="raw", bufs=3)
    ar_pool = tc.alloc_tile_pool(name="ar", bufs=2)
    ps_sc = tc.alloc_tile_pool(name="pssc", bufs=2, space="PSUM")
    ps_po = tc.alloc_tile_pool(name="pspo", bufs=2, space="PSUM")

    for b in range(BL):
        et = et_pool.tile([128, 2, H, N], BF16, tag="et")
        ar = ar_pool.tile([128, H, N], F32, tag="ar")
        for hg in range(3):
            pss = []
            for kt in range(2):
                nk = NK0 if kt == 0 else NK1
                kofs = b * N + kt * 128
                ps = ps_sc.tile([128, 4, 256], F32, tag="pssc")
                pss.append(ps)
                for j in range(4):
                    h = hg * 4 + j
                    off = (h % 2) * 64
                    # scoresT[nk, nq] = kT.T @ qT  (scale folded into Wq);
                    # psum slice SIG4[j] so concurrent row-packed MMs use
                    # different banks
                    nc.tensor.matmul(
                        ps[:nk, SIG4[j], 0:N],
                        qkT[6 + h // 2][off : off + 64, kofs : kofs + nk],
                        qkT[h // 2][off : off + 64, b * N : b * N + N],
                        start=True,
                        stop=True,
                    )
                raw = raw_pool.tile([128, 4, N], BF16, tag="raw")
                nc.scalar.activation(raw[:nk, :, :], ps[:nk, :, 0:N], exp)
                # multiplicative relative-position bias: et = exp(z)*exp(bias)
                nc.vector.tensor_mul(
                    et[:nk, kt, hg * 4 : (hg + 1) * 4, :],
                    raw[:nk, :, :],
                    bias_sb[kt][:nk, hg * 4 : (hg + 1) * 4, :],
                )
                # softmax denominator: a ones-row matmul both reduces
                # across partitions and broadcasts the result to all 128
                # output partitions; both nk tiles accumulate into the
                # kt-0 scores psum (reusing it after exp consumed it)
                flat = pss[0].rearrange("p a b -> p (a b)")
                for pr in range(2):
                    nc.tensor.matmul(
                        flat[:, pr * 512 : pr * 512 + 2 * N],
                        ones128[:nk, :],
                        et[:nk, kt, hg * 4 + 2 * pr : hg * 4 + 2 * pr + 2, :],
                        start=(kt == 0),
                        stop=(kt == 1),
                    )
            # reciprocal of the denominators (identical on every partition)
            flat = pss[0].rearrange("p a b -> p (a b)")
            for pr in range(2):
                nc.vector.reciprocal_approx_fast(
                    out=ar[:, hg * 4 + 2 * pr : hg * 4 + 2 * pr + 2, :],
                    in_=flat[:, pr * 512 : pr * 512 + 2 * N],
                )

        for grp in range(3):
            po = ps_po.tile([128, 2, 256], F32, tag="pspo")
            for j in range(4):
                h = grp * 4 + j
                base = (j % 2) * 64
                sl = j // 2
                for kt in range(2):
                    nk = NK0 if kt == 0 else NK1
                    nc.tensor.matmul(
                        po[base : base + 64, sl, 0:N],
                        v_sb[b][kt][:nk, h * 64 : (h + 1) * 64],
                        et[:nk, kt, sig(h), :],
                        start=(kt == 0),
                        stop=(kt == 1),
                        tile_position=(0, base),
                    )
            for j in range(4):
                h = grp * 4 + j
                base = (j % 2) * 64
                sl = j // 2
                nc.vector.tensor_tensor(
                    aoT[h // 2][base : base + 64, b * N : b * N + N],
                    po[base : base + 64, sl, 0:N],
                    ar[base : base + 64, sig(h), :],
                    mybir.AluOpType.mult,
                )

    # ---- phase 3: y = attn_outT.T @ projT + proj_b ----
    with tc.tile_pool(name="ostg", bufs=3) as ostg_pool:
        for mt in range(MT3):
            msz = 128 if mt < MT3 - 1 else M - 128 * (MT3 - 1)
            stg = ostg_pool.tile([128, C], F32, tag="stg")
            for nt2 in range(2):
                ns = slice(nt2 * 384, (nt2 + 1) * 384)
                ps = ps_mm.tile([128, 384], F32, tag="ps1")
                for c in range(KC):
                    nc.tensor.matmul(
                        ps[:msz, :],
                        aoT[c][:, mt * 128 : mt * 128 + msz],
                        wp[c][:, ns],
                        start=(c == 0),
                        stop=(c == KC - 1),
                    )
                nc.any.tensor_add(stg[:msz, ns], ps[:msz, :], projb_bc[:msz, ns])
            nc.sync.dma_start(
                out=out_d[mt * 128 : mt * 128 + msz, :], in_=stg[:msz, :]
            )

    for pool in (
        ps_po,
        ps_sc,
        ar_pool,
        raw_pool,
        et_pool,
        ao_pool,
        wp_pool,
        ps_mm,
        v_pool,
        qk_pool,
        consts,
    ):
        pool.release()


def _get_compiled():
    if "nc" not in _COMPILED:
        _COMPILED["nc"] = _build_nc()
    return _COMPILED["nc"]


def _prep_host(inputs):
    qkv_w = np.asarray(inputs["qkv_w"], dtype=np.float32)
    proj_w = np.asarray(inputs["proj_w"], dtype=np.float32)
    proj_b = np.asarray(inputs["proj_b"], dtype=np.float32)
    rel_table = np.asarray(inputs["rel_table"], dtype=np.float32)
    rel_index = np.asarray(inputs["rel_index"]).astype(np.int64)

    w = qkv_w.copy()
    w[:C] *= SCALE  # fold the attention scale into Wq
    wqkvT = np.ascontiguousarray(w.T)
    wprojT = np.ascontiguousarray(proj_w.T).astype(ml_dtypes.bfloat16)
    projb2 = np.ascontiguousarray(proj_b.reshape(1, C))

    bias_full = rel_table[rel_index]          # [nq, nk, H]
    biasT = np.exp(bias_full.transpose(2, 1, 0))  # [H, nk, nq], exp for the
    # multiplicative-bias trick: exp(z + b) = exp(z) * exp(b)
    perm = [(t // 4) * 4 + (0, 2, 1, 3)[t % 4] for t in range(H)]
    biasT = biasT[perm]
    pad = np.zeros((H, 2 * 128, N), np.float32)
    pad[:, :N, :] = biasT
    bias_dev = np.ascontiguousarray(
        pad.reshape(H, 2, 128, N).transpose(1, 2, 0, 3)
    ).astype(ml_dtypes.bfloat16)
    return wqkvT, wprojT, projb2, bias_dev


def kernel(**inputs):
    x = np.asarray(inputs["x"], dtype=np.float32)
    wqkvT, wprojT, projb2, bias_dev = _prep_host(inputs)

    nc = _get_compiled()
    in_maps = []
    for i in range(N_CORES):
        shard = x[i * BL : (i + 1) * BL].reshape(M, C)
        in_maps.append(
            {
                "xT": np.ascontiguousarray(shard.T),
                "wqkvT": wqkvT,
                "wprojT": wprojT,
                "projb": projb2,
                "biasT": bias_dev,
            }
        )
    res = run_bass_kernel_spmd(nc, in_maps, core_ids=list(range(N_CORES)))
    out = np.empty((B, N, C), dtype=np.float32)
    for i in range(N_CORES):
        out[i * BL : (i + 1) * BL] = res.results[i]["out"].reshape(BL, N, C)
    return out


def run_traced(**inputs):
    """Like kernel() but with NTFF tracing; returns (out, BassKernelResults)."""
    x = np.asarray(inputs["x"], dtype=np.float32)
    wqkvT, wprojT, projb2, bias_dev = _prep_host(inputs)
    nc = _get_compiled()
    in_maps = []
    for i in range(N_CORES):
        shard = x[i * BL : (i + 1) * BL].reshape(M, C)
        in_maps.append(
            {
                "xT": np.ascontiguousarray(shard.T),
                "wqkvT": wqkvT,
                "wprojT": wprojT,
                "projb": projb2,
                "biasT": bias_dev,
            }
        )
    res = run_bass_kernel_spmd(
        nc, in_maps, core_ids=list(range(N_CORES)), trace=True
    )
    out = np.empty((B, N, C), dtype=np.float32)
    for i in range(N_CORES):
        out[i * BL : (i + 1) * BL] = res.results[i]["out"].reshape(BL, N, C)
    return out, res


# revision 23
# speedup vs baseline: 1.8947x; 1.1122x over previous
"""Trainium2 Bass kernel for a 12-head dense attention block (BEiT-style
windowed attention with relative-position bias), batch-parallel over 8
NeuronCores.

Shapes (hardcoded): x [64, 197, 768], qkv_w [2304, 768], proj_w [768, 768],
proj_b [768], rel_table [732, 12], rel_index [197, 197] int32.

Sharding: data-parallel over batch — each of the 8 cores handles 8 batch
elements end-to-end; no collectives. Host pre-transposes x and the weights
so the device kernel needs no on-chip transposes:

  phase 1: qkT[2C, M] = wqkvT.T-style matmul producing q,k TRANSPOSED
           ([feature, token]) + v in natural layout ([token, feature]),
           bf16 matmuls (fast weight load).
  phase 2: per (batch, head): scoresT[nk, nq] = kT.T @ qT, exp on the
           scalar engine, relative-position bias applied multiplicatively
           (exp(bias) precomputed), softmax denominator via gpsimd
           partition_all_reduce, attention output accumulated TRANSPOSED
           (outT[d, nq] = v.T-free matmul) and normalized by a DVE multiply.
  phase 3: y = attn_outT.T @ projT (bf16) + broadcast bias add,
           DMA out in natural layout.
"""

import sys

if "/opt/trn_rl_repo" not in sys.path:
    sys.path.insert(0, "/opt/trn_rl_repo")

import numpy as np
import ml_dtypes

import concourse.bass as bass  # noqa: F401  (registers rust bindings)
import concourse.tile as tile
from concourse import bacc, bass_isa, mybir
from concourse.bass_utils import run_bass_kernel_spmd

N_CORES = 8
B, N, C, H, D = 64, 197, 768, 12, 64
BL = B // N_CORES            # 8 batch elements per core
M = BL * N                   # 1576 tokens per core
SCALE = D ** -0.5
NK0 = 128
NK1 = N - NK0                # 69
KC = C // 128                # 6 contraction chunks
MT = 4                       # m-tiles in phase 1 (qk part)
MTS = M // MT                # 394
NT_QK = (2 * C) // 128       # 12 output-feature tiles for q,k
MT3 = (M + 127) // 128       # 13 m-tiles in phase 3

F32 = mybir.dt.float32
BF16 = mybir.dt.bfloat16
F32R = mybir.dt.float32r

# self-inverse head<->slot permutation within each 4-head group: consecutive
# score matmuls alternate array row-strips (head parity) and run concurrently,
# so they must target different PSUM banks -> interleave slots (0,2,1,3)
SIG4 = (0, 2, 1, 3)


def sig(h):
    return (h // 4) * 4 + SIG4[h % 4]


_COMPILED = {}


def _build_nc():
    nc = bacc.Bacc(
        "TRN2", target_bir_lowering=False, debug=False, num_devices=N_CORES
    )
    xT = nc.declare_dram_parameter("xT", [C, M], BF16, isOutput=False)
    wqkvT = nc.declare_dram_parameter("wqkvT", [C, 3 * C], BF16, isOutput=False)
    wprojT = nc.declare_dram_parameter("wprojT", [C, C], BF16, isOutput=False)
    projb = nc.declare_dram_parameter("projb", [1, C], F32, isOutput=False)
    biasT = nc.declare_dram_parameter("biasT", [2, 128, H, N], BF16, isOutput=False)
    out_d = nc.declare_dram_parameter("out", [M, C], F32, isOutput=True)

    with tile.TileContext(nc) as tc:
        _body(nc, tc, xT, wqkvT, wprojT, projb, biasT, out_d)
    nc.compile()
    return nc


def _body(nc, tc, xT, wqkvT, wprojT, projb, biasT, out_d):
    exp = mybir.ActivationFunctionType.Exp

    consts = tc.alloc_tile_pool(name="consts", bufs=1)
    ones128 = consts.tile([128, 128], BF16)
    nc.vector.memset(ones128, 1.0)
    projb_sb = consts.tile([1, C], F32)
    nc.sync.dma_start(out=projb_sb[:, :], in_=projb[:, :])
    projb_bc = consts.tile([128, C], F32)
    _pb = projb[:, :]
    nc.sync.dma_start(
        out=projb_bc[:, :],
        in_=bass.AP(tensor=_pb.tensor, offset=_pb.offset, ap=[[0, 128], [1, C]]),
    )
    bias_sb = [consts.tile([128, H, N], BF16, tag=f"bias{t}", name=f"bias{t}") for t in range(2)]
    for t in range(2):
        nc.sync.dma_start(out=bias_sb[t][:, :, :], in_=biasT[t, :, :, :])

    # ---- outputs of phase 1 (persist into phase 2) ----
    qk_pool = tc.alloc_tile_pool(name="qk", bufs=1)
    qkT = [qk_pool.tile([128, M], BF16, tag=f"qk{t}", name=f"qk{t}") for t in range(NT_QK)]
    v_pool = tc.alloc_tile_pool(name="v", bufs=1)
    v_sb = [
        [v_pool.tile([128, C], BF16, tag=f"v{b}_{pt}", name=f"v{b}_{pt}") for pt in range(2)]
        for b in range(BL)
    ]

    ps_mm = tc.alloc_tile_pool(name="psmm", bufs=2, space="PSUM")

    # ---- phase 1: qkT (transposed q,k) and v (natural) ----
    with tc.tile_pool(name="xt", bufs=1) as xt_pool, tc.tile_pool(
        name="wq", bufs=1
    ) as wq_pool:
        xt = [xt_pool.tile([128, M], BF16, tag=f"xt{k}", name=f"xt{k}") for k in range(KC)]
        wq = [wq_pool.tile([128, 3 * C], BF16, tag=f"wq{k}", name=f"wq{k}") for k in range(KC)]
        for k in range(KC):
            nc.sync.dma_start(out=xt[k][:, :], in_=xT[k * 128 : (k + 1) * 128, :])
            nc.sync.dma_start(
                out=wq[k][:, :], in_=wqkvT[k * 128 : (k + 1) * 128, :]
            )

        for mt in range(MT):
            ms = slice(mt * MTS, (mt + 1) * MTS)
            for nt in range(NT_QK):
                ps = ps_mm.tile([128, MTS], F32, tag="ps1")
                for k in range(KC):
                    nc.tensor.matmul(
                        ps[:, :],
                        wq[k][:, nt * 128 : (nt + 1) * 128],
                        xt[k][:, ms],
                        start=(k == 0),
                        stop=(k == KC - 1),
                    )
                nc.any.tensor_copy(qkT[nt][:, ms], ps[:, :])
            for b in (2 * mt, 2 * mt + 1):
                for pt in range(2):
                    psz = NK0 if pt == 0 else NK1
                    mofs = b * N + pt * 128
                    for nt2 in range(2):
                        ps = ps_mm.tile([128, 384], F32, tag="ps1")
                        for k in range(KC):
                            nc.tensor.matmul(
                                ps[:psz, :],
                                xt[k][:, mofs : mofs + psz],
                                wq[k][
                                    :, 2 * C + nt2 * 384 : 2 * C + (nt2 + 1) * 384
                                ],
                                start=(k == 0),
                                stop=(k == KC - 1),
                            )
                        nc.any.tensor_copy(
                            v_sb[b][pt][:psz, nt2 * 384 : (nt2 + 1) * 384],
                            ps[:psz, :],
                        )

    # ---- phase 3 weights: load early into space freed by xt/wq ----
    wp_pool = tc.alloc_tile_pool(name="wp", bufs=1)
    wp = [wp_pool.tile([128, C], BF16, tag=f"wp{k}", name=f"wp{k}") for k in range(KC)]
    for k in range(KC):
        nc.sync.dma_start(out=wp[k][:, :], in_=wprojT[k * 128 : (k + 1) * 128, :])

    ao_pool = tc.alloc_tile_pool(name="ao", bufs=1)
    aoT = [ao_pool.tile([128, M], BF16, tag=f"ao{t}", name=f"ao{t}") for t in range(KC)]

    # ---- phase 2: attention per batch element ----
    et_pool = tc.alloc_tile_pool(name="et", bufs=2)
    raw_pool = tc.alloc_tile_pool(name="raw", bufs=3)
    ar_pool = tc.alloc_tile_pool(name="ar", bufs=2)
    ps_sc = tc.alloc_tile_pool(name="pssc", bufs=2, space="PSUM")
    ps_po = tc.alloc_tile_pool(name="pspo", bufs=2, space="PSUM")

    for b in range(BL):
        et = et_pool.tile([128, 2, H, N], BF16, tag="et")
        ar = ar_pool.tile([128, H, N], F32, tag="ar")
        for hg in range(3):
            pss = []
            for kt in range(2):
                nk = NK0 if kt == 0 else NK1
                kofs = b * N + kt * 128
                ps = ps_sc.tile([128, 4, 256], F32, tag="pssc")
                pss.append(ps)
                for j in range(4):
                    h = hg * 4 + j
                    off = (h % 2) * 64
                    # scoresT[nk, nq] = kT.T @ qT  (scale folded into Wq);
                    # psum slice SIG4[j] so concurrent row-packed MMs use
                    # different banks
                    nc.tensor.matmul(
                        ps[:nk, SIG4[j], 0:N],
                        qkT[6 + h // 2][off : off + 64, kofs : kofs + nk],
                        qkT[h // 2][off : off + 64, b * N : b * N + N],
                        start=True,
                        stop=True,
                    )
                raw = raw_pool.tile([128, 4, N], BF16, tag="raw")
                nc.scalar.activation(raw[:nk, :, :], ps[:nk, :, 0:N], exp)
                # multiplicative relative-position bias: et = exp(z)*exp(bias)
                nc.vector.tensor_mul(
                    et[:nk, kt, hg * 4 : (hg + 1) * 4, :],
                    raw[:nk, :, :],
                    bias_sb[kt][:nk, hg * 4 : (hg + 1) * 4, :],
                )
                # softmax denominator: a ones-row matmul both reduces
                # across partitions and broadcasts the result to all 128
                # output partitions; both nk tiles accumulate into the
                # kt-0 scores psum (reusing it after exp consumed it)
                flat = pss[0].rearrange("p a b -> p (a b)")
                for pr in range(2):
                    nc.tensor.matmul(
                        flat[:, pr * 512 : pr * 512 + 2 * N],
                        ones128[:nk, :],
                        et[:nk, kt, hg * 4 + 2 * pr : hg * 4 + 2 * pr + 2, :],
                        start=(kt == 0),
                        stop=(kt == 1),
                    )
            # reciprocal of the denominators (identical on every partition)
            flat = pss[0].rearrange("p a b -> p (a b)")
            for pr in range(2):
                nc.vector.reciprocal_approx_fast(
                    out=ar[:, hg * 4 + 2 * pr : hg * 4 + 2 * pr + 2, :],
                    in_=flat[:, pr * 512 : pr * 512 + 2 * N],
                )

        for grp in range(3):
            po = ps_po.tile([128, 2, 256], F32, tag="pspo")
            for j in range(4):
                h = grp * 4 + j
                base = (j % 2) * 64
                sl = j // 2
                for kt in range(2):
                    nk = NK0 if kt == 0 else NK1
                    nc.tensor.matmul(
                        po[base : base + 64, sl, 0:N],
                        v_sb[b][kt][:nk, h * 64 : (h + 1) * 64],
                        et[:nk, kt, sig(h), :],
                        start=(kt == 0),
                        stop=(kt == 1),
                        tile_position=(0, base),
                    )
            for j in range(4):
                h = grp * 4 + j
                base = (j % 2) * 64
                sl = j // 2
                nc.vector.tensor_tensor(
                    aoT[h // 2][base : base + 64, b * N : b * N + N],
                    po[base : base + 64, sl, 0:N],
                    ar[base : base + 64, sig(h), :],
                    mybir.AluOpType.mult,
                )

    # ---- phase 3: y = attn_outT.T @ projT + proj_b ----
    with tc.tile_pool(name="ostg", bufs=3) as ostg_pool:
        for mt in range(MT3):
            msz = 128 if mt < MT3 - 1 else M - 128 * (MT3 - 1)
            stg = ostg_pool.tile([128, C], F32, tag="stg")
            for nt2 in range(2):
                ns = slice(nt2 * 384, (nt2 + 1) * 384)
                ps = ps_mm.tile([128, 384], F32, tag="ps1")
                for c in range(KC):
                    nc.tensor.matmul(
                        ps[:msz, :],
                        aoT[c][:, mt * 128 : mt * 128 + msz],
                        wp[c][:, ns],
                        start=(c == 0),
                        stop=(c == KC - 1),
                    )
                nc.any.tensor_add(stg[:msz, ns], ps[:msz, :], projb_bc[:msz, ns])
            nc.sync.dma_start(
                out=out_d[mt * 128 : mt * 128 + msz, :], in_=stg[:msz, :]
            )

    for pool in (
        ps_po,
        ps_sc,
        ar_pool,
        raw_pool,
        et_pool,
        ao_pool,
        wp_pool,
        ps_mm,
        v_pool,
        qk_pool,
        consts,
    ):
        pool.release()


def _get_compiled():
    if "nc" not in _COMPILED:
        _COMPILED["nc"] = _build_nc()
    return _COMPILED["nc"]


def _prep_host(inputs):
    qkv_w = np.asarray(inputs["qkv_w"], dtype=np.float32)
    proj_w = np.asarray(inputs["proj_w"], dtype=np.float32)
    proj_b = np.asarray(inputs["proj_b"], dtype=np.float32)
    rel_table = np.asarray(inputs["rel_table"], dtype=np.float32)
    rel_index = np.asarray(inputs["rel_index"]).astype(np.int64)

    w = qkv_w.copy()
    w[:C] *= SCALE  # fold the attention scale into Wq
    wqkvT = np.ascontiguousarray(w.T).astype(ml_dtypes.bfloat16)
    wprojT = np.ascontiguousarray(proj_w.T).astype(ml_dtypes.bfloat16)
    projb2 = np.ascontiguousarray(proj_b.reshape(1, C))

    bias_full = rel_table[rel_index]          # [nq, nk, H]
    biasT = np.exp(bias_full.transpose(2, 1, 0))  # [H, nk, nq], exp for the
    # multiplicative-bias trick: exp(z + b) = exp(z) * exp(b)
    perm = [(t // 4) * 4 + (0, 2, 1, 3)[t % 4] for t in range(H)]
    biasT = biasT[perm]
    pad = np.zeros((H, 2 * 128, N), np.float32)
    pad[:, :N, :] = biasT
    bias_dev = np.ascontiguousarray(
        pad.reshape(H, 2, 128, N).transpose(1, 2, 0, 3)
    ).astype(ml_dtypes.bfloat16)
    return wqkvT, wprojT, projb2, bias_dev


def kernel(**inputs):
    x = np.asarray(inputs["x"], dtype=np.float32)
    wqkvT, wprojT, projb2, bias_dev = _prep_host(inputs)

    nc = _get_compiled()
    in_maps = []
    for i in range(N_CORES):
        shard = x[i * BL : (i + 1) * BL].reshape(M, C)
        in_maps.append(
            {
                "xT": np.ascontiguousarray(shard.T).astype(ml_dtypes.bfloat16),
                "wqkvT": wqkvT,
                "wprojT": wprojT,
                "projb": projb2,
                "biasT": bias_dev,
            }
        )
    res = run_bass_kernel_spmd(nc, in_maps, core_ids=list(range(N_CORES)))
    out = np.empty((B, N, C), dtype=np.float32)
    for i in range(N_CORES):
        out[i * BL : (i + 1) * BL] = res.results[i]["out"].reshape(BL, N, C)
    return out


def run_traced(**inputs):
    """Like kernel() but with NTFF tracing; returns (out, BassKernelResults)."""
    x = np.asarray(inputs["x"], dtype=np.float32)
    wqkvT, wprojT, projb2, bias_dev = _prep_host(inputs)
    nc = _get_compiled()
    in_maps = []
    for i in range(N_CORES):
        shard = x[i * BL : (i + 1) * BL].reshape(M, C)
        in_maps.append(
            {
                "xT": np.ascontiguousarray(shard.T).astype(ml_dtypes.bfloat16),
                "wqkvT": wqkvT,
                "wprojT": wprojT,
                "projb": projb2,
                "biasT": bias_dev,
            }
        )
    res = run_bass_kernel_spmd(
        nc, in_maps, core_ids=list(range(N_CORES)), trace=True
    )
    out = np.empty((B, N, C), dtype=np.float32)
    for i in range(N_CORES):
        out[i * BL : (i + 1) * BL] = res.results[i]["out"].reshape(BL, N, C)
    return out, res


# revision 25
# speedup vs baseline: 1.9232x; 1.0151x over previous
"""Trainium2 Bass kernel for a 12-head dense attention block (BEiT-style
windowed attention with relative-position bias), batch-parallel over 8
NeuronCores.

Shapes (hardcoded): x [64, 197, 768], qkv_w [2304, 768], proj_w [768, 768],
proj_b [768], rel_table [732, 12], rel_index [197, 197] int32.

Sharding: data-parallel over batch — each of the 8 cores handles 8 batch
elements end-to-end; no collectives. Host pre-transposes x and the weights
so the device kernel needs no on-chip transposes:

  phase 1: qkT[2C, M] = wqkvT.T-style matmul producing q,k TRANSPOSED
           ([feature, token]) + v in natural layout ([token, feature]),
           bf16 matmuls (fast weight load).
  phase 2: per (batch, head): scoresT[nk, nq] = kT.T @ qT, exp on the
           scalar engine, relative-position bias applied multiplicatively
           (exp(bias) precomputed), softmax denominator via gpsimd
           partition_all_reduce, attention output accumulated TRANSPOSED
           (outT[d, nq] = v.T-free matmul) and normalized by a DVE multiply.
  phase 3: y = attn_outT.T @ projT (bf16) + broadcast bias add,
           DMA out in natural layout.
"""

import sys

if "/opt/trn_rl_repo" not in sys.path:
    sys.path.insert(0, "/opt/trn_rl_repo")

import numpy as np
import ml_dtypes

import concourse.bass as bass  # noqa: F401  (registers rust bindings)
import concourse.tile as tile
from concourse import bacc, bass_isa, mybir
from concourse.bass_utils import run_bass_kernel_spmd

N_CORES = 8
B, N, C, H, D = 64, 197, 768, 12, 64
BL = B // N_CORES            # 8 batch elements per core
M = BL * N                   # 1576 tokens per core
SCALE = D ** -0.5
NK0 = 128
NK1 = N - NK0                # 69
KC = C // 128                # 6 contraction chunks
MT = 4                       # m-tiles in phase 1 (qk part)
MTS = M // MT                # 394
NT_QK = (2 * C) // 128       # 12 output-feature tiles for q,k
MT3 = (M + 127) // 128       # 13 m-tiles in phase 3

F32 = mybir.dt.float32
BF16 = mybir.dt.bfloat16
F32R = mybir.dt.float32r

# self-inverse head<->slot permutation within each 4-head group: consecutive
# score matmuls alternate array row-strips (head parity) and run concurrently,
# so they must target different PSUM banks -> interleave slots (0,2,1,3)
SIG4 = (0, 2, 1, 3)


def sig(h):
    return (h // 4) * 4 + SIG4[h % 4]


_COMPILED = {}


def _build_nc():
    nc = bacc.Bacc(
        "TRN2", target_bir_lowering=False, debug=False, num_devices=N_CORES
    )
    xT = nc.declare_dram_parameter("xT", [C, M], BF16, isOutput=False)
    wqkvT = nc.declare_dram_parameter("wqkvT", [C, 3 * C], BF16, isOutput=False)
    wprojT = nc.declare_dram_parameter("wprojT", [C, C], BF16, isOutput=False)
    projb = nc.declare_dram_parameter("projb", [1, C], F32, isOutput=False)
    biasT = nc.declare_dram_parameter("biasT", [2, 128, H, N], BF16, isOutput=False)
    out_d = nc.declare_dram_parameter("out", [M, C], F32, isOutput=True)

    with tile.TileContext(nc) as tc:
        _body(nc, tc, xT, wqkvT, wprojT, projb, biasT, out_d)
    nc.compile()
    return nc


def _body(nc, tc, xT, wqkvT, wprojT, projb, biasT, out_d):
    exp = mybir.ActivationFunctionType.Exp

    consts = tc.alloc_tile_pool(name="consts", bufs=1)
    ones128 = consts.tile([128, 128], BF16)
    nc.vector.memset(ones128, 1.0)
    projb_sb = consts.tile([1, C], F32)
    nc.sync.dma_start(out=projb_sb[:, :], in_=projb[:, :])
    projb_bc = consts.tile([128, C], F32)
    bias_sb = [consts.tile([128, H, N], BF16, tag=f"bias{t}", name=f"bias{t}") for t in range(2)]
    deferred_dmas = []

    # ---- outputs of phase 1 (persist into phase 2) ----
    qk_pool = tc.alloc_tile_pool(name="qk", bufs=1)
    qkT = [qk_pool.tile([128, M], BF16, tag=f"qk{t}", name=f"qk{t}") for t in range(NT_QK)]
    v_pool = tc.alloc_tile_pool(name="v", bufs=1)
    v_sb = [
        [v_pool.tile([128, C], BF16, tag=f"v{b}_{pt}", name=f"v{b}_{pt}") for pt in range(2)]
        for b in range(BL)
    ]

    ps_mm = tc.alloc_tile_pool(name="psmm", bufs=2, space="PSUM")

    # ---- phase 1: qkT (transposed q,k) and v (natural) ----
    with tc.tile_pool(name="xt", bufs=1) as xt_pool, tc.tile_pool(
        name="wq", bufs=1
    ) as wq_pool:
        xt = [xt_pool.tile([128, M], BF16, tag=f"xt{k}", name=f"xt{k}") for k in range(KC)]
        wq = [wq_pool.tile([128, 3 * C], BF16, tag=f"wq{k}", name=f"wq{k}") for k in range(KC)]
        for k in range(KC):
            nc.sync.dma_start(out=xt[k][:, :], in_=xT[k * 128 : (k + 1) * 128, :])
            last_in_dma = nc.sync.dma_start(
                out=wq[k][:, :], in_=wqkvT[k * 128 : (k + 1) * 128, :]
            )
        # secondary inputs (bias table, proj bias broadcast) wait for the
        # phase-1 inputs so the startup DMA ramp is as short as possible
        _pb = projb[:, :]
        deferred_dmas.append(
            nc.sync.dma_start(
                out=projb_bc[:, :],
                in_=bass.AP(
                    tensor=_pb.tensor, offset=_pb.offset, ap=[[0, 128], [1, C]]
                ),
            )
        )
        for t in range(2):
            deferred_dmas.append(
                nc.sync.dma_start(out=bias_sb[t][:, :, :], in_=biasT[t, :, :, :])
            )
        for d in deferred_dmas:
            tile.add_dep_helper(d.ins, last_in_dma.ins, sync=True, reason="defer-input")

        for mt in range(MT):
            ms = slice(mt * MTS, (mt + 1) * MTS)
            for nt in range(NT_QK):
                ps = ps_mm.tile([128, MTS], F32, tag="ps1")
                for k in range(KC):
                    nc.tensor.matmul(
                        ps[:, :],
                        wq[k][:, nt * 128 : (nt + 1) * 128],
                        xt[k][:, ms],
                        start=(k == 0),
                        stop=(k == KC - 1),
                    )
                nc.any.tensor_copy(qkT[nt][:, ms], ps[:, :])
            for b in (2 * mt, 2 * mt + 1):
                for pt in range(2):
                    psz = NK0 if pt == 0 else NK1
                    mofs = b * N + pt * 128
                    for nt2 in range(2):
                        ps = ps_mm.tile([128, 384], F32, tag="ps1")
                        for k in range(KC):
                            nc.tensor.matmul(
                                ps[:psz, :],
                                xt[k][:, mofs : mofs + psz],
                                wq[k][
                                    :, 2 * C + nt2 * 384 : 2 * C + (nt2 + 1) * 384
                                ],
                                start=(k == 0),
                                stop=(k == KC - 1),
                            )
                        nc.any.tensor_copy(
                            v_sb[b][pt][:psz, nt2 * 384 : (nt2 + 1) * 384],
                            ps[:psz, :],
                        )

    # ---- phase 3 weights: load early into space freed by xt/wq ----
    wp_pool = tc.alloc_tile_pool(name="wp", bufs=1)
    wp = [wp_pool.tile([128, C], BF16, tag=f"wp{k}", name=f"wp{k}") for k in range(KC)]
    for k in range(KC):
        _d = nc.sync.dma_start(
            out=wp[k][:, :], in_=wprojT[k * 128 : (k + 1) * 128, :]
        )
        tile.add_dep_helper(_d.ins, last_in_dma.ins, sync=True, reason="defer-wp")

    ao_pool = tc.alloc_tile_pool(name="ao", bufs=1)
    aoT = [ao_pool.tile([128, M], BF16, tag=f"ao{t}", name=f"ao{t}") for t in range(KC)]

    # ---- phase 2: attention per batch element ----
    et_pool = tc.alloc_tile_pool(name="et", bufs=2)
    raw_pool = tc.alloc_tile_pool(name="raw", bufs=3)
    ar_pool = tc.alloc_tile_pool(name="ar", bufs=2)
    ps_sc = tc.alloc_tile_pool(name="pssc", bufs=2, space="PSUM")
    ps_po = tc.alloc_tile_pool(name="pspo", bufs=2, space="PSUM")

    for b in range(BL):
        et = et_pool.tile([128, 2, H, N], BF16, tag="et")
        ar = ar_pool.tile([128, H, N], F32, tag="ar")
        for hg in range(3):
            pss = []
            for kt in range(2):
                nk = NK0 if kt == 0 else NK1
                kofs = b * N + kt * 128
                ps = ps_sc.tile([128, 4, 256], F32, tag="pssc")
                pss.append(ps)
                for j in range(4):
                    h = hg * 4 + j
                    off = (h % 2) * 64
                    # scoresT[nk, nq] = kT.T @ qT  (scale folded into Wq);
                    # psum slice SIG4[j] so concurrent row-packed MMs use
                    # different banks
                    nc.tensor.matmul(
                        ps[:nk, SIG4[j], 0:N],
                        qkT[6 + h // 2][off : off + 64, kofs : kofs + nk],
                        qkT[h // 2][off : off + 64, b * N : b * N + N],
                        start=True,
                        stop=True,
                    )
                raw = raw_pool.tile([128, 4, N], BF16, tag="raw")
                nc.scalar.activation(raw[:nk, :, :], ps[:nk, :, 0:N], exp)
                # multiplicative relative-position bias: et = exp(z)*exp(bias)
                nc.vector.tensor_mul(
                    et[:nk, kt, hg * 4 : (hg + 1) * 4, :],
                    raw[:nk, :, :],
                    bias_sb[kt][:nk, hg * 4 : (hg + 1) * 4, :],
                )
                # softmax denominator: a ones-row matmul both reduces
                # across partitions and broadcasts the result to all 128
                # output partitions; both nk tiles accumulate into the
                # kt-0 scores psum (reusing it after exp consumed it)
                flat = pss[0].rearrange("p a b -> p (a b)")
                for pr in range(2):
                    nc.tensor.matmul(
                        flat[:, pr * 512 : pr * 512 + 2 * N],
                        ones128[:nk, :],
                        et[:nk, kt, hg * 4 + 2 * pr : hg * 4 + 2 * pr + 2, :],
                        start=(kt == 0),
                        stop=(kt == 1),
                    )
            # reciprocal of the denominators (identical on every partition)
            flat = pss[0].rearrange("p a b -> p (a b)")
            for pr in range(2):
                nc.vector.reciprocal_approx_fast(
                    out=ar[:, hg * 4 + 2 * pr : hg * 4 + 2 * pr + 2, :],
                    in_=flat[:, pr * 512 : pr * 512 + 2 * N],
                )

        for grp in range(3):
            po = ps_po.tile([128, 2, 256], F32, tag="pspo")
            for j in range(4):
                h = grp * 4 + j
                base = (j % 2) * 64
                sl = j // 2
                for kt in range(2):
                    nk = NK0 if kt == 0 else NK1
                    nc.tensor.matmul(
                        po[base : base + 64, sl, 0:N],
                        v_sb[b][kt][:nk, h * 64 : (h + 1) * 64],
                        et[:nk, kt, sig(h), :],
                        start=(kt == 0),
                        stop=(kt == 1),
                        tile_position=(0, base),
                    )
            for j in range(4):
                h = grp * 4 + j
                base = (j % 2) * 64
                sl = j // 2
                nc.vector.tensor_tensor(
                    aoT[h // 2][base : base + 64, b * N : b * N + N],
                    po[base : base + 64, sl, 0:N],
                    ar[base : base + 64, sig(h), :],
                    mybir.AluOpType.mult,
                )

    # ---- phase 3: y = attn_outT.T @ projT + proj_b ----
    with tc.tile_pool(name="ostg", bufs=3) as ostg_pool:
        for mt in range(MT3):
            msz = 128 if mt < MT3 - 1 else M - 128 * (MT3 - 1)
            stg = ostg_pool.tile([128, C], F32, tag="stg")
            for nt2 in range(2):
                ns = slice(nt2 * 384, (nt2 + 1) * 384)
                ps = ps_mm.tile([128, 384], F32, tag="ps1")
                for c in range(KC):
                    nc.tensor.matmul(
                        ps[:msz, :],
                        aoT[c][:, mt * 128 : mt * 128 + msz],
                        wp[c][:, ns],
                        start=(c == 0),
                        stop=(c == KC - 1),
                    )
                nc.any.tensor_add(stg[:msz, ns], ps[:msz, :], projb_bc[:msz, ns])
            nc.sync.dma_start(
                out=out_d[mt * 128 : mt * 128 + msz, :], in_=stg[:msz, :]
            )

    for pool in (
        ps_po,
        ps_sc,
        ar_pool,
        raw_pool,
        et_pool,
        ao_pool,
        wp_pool,
        ps_mm,
        v_pool,
        qk_pool,
        consts,
    ):
        pool.release()


def _get_compiled():
    if "nc" not in _COMPILED:
        _COMPILED["nc"] = _build_nc()
    return _COMPILED["nc"]


def _prep_host(inputs):
    qkv_w = np.asarray(inputs["qkv_w"], dtype=np.float32)
    proj_w = np.asarray(inputs["proj_w"], dtype=np.float32)
    proj_b = np.asarray(inputs["proj_b"], dtype=np.float32)
    rel_table = np.asarray(inputs["rel_table"], dtype=np.float32)
    rel_index = np.asarray(inputs["rel_index"]).astype(np.int64)

    w = qkv_w.copy()
    w[:C] *= SCALE  # fold the attention scale into Wq
    wqkvT = np.ascontiguousarray(w.T).astype(ml_dtypes.bfloat16)
    wprojT = np.ascontiguousarray(proj_w.T).astype(ml_dtypes.bfloat16)
    projb2 = np.ascontiguousarray(proj_b.reshape(1, C))

    bias_full = rel_table[rel_index]          # [nq, nk, H]
    biasT = np.exp(bias_full.transpose(2, 1, 0))  # [H, nk, nq], exp for the
    # multiplicative-bias trick: exp(z + b) = exp(z) * exp(b)
    perm = [(t // 4) * 4 + (0, 2, 1, 3)[t % 4] for t in range(H)]
    biasT = biasT[perm]
    pad = np.zeros((H, 2 * 128, N), np.float32)
    pad[:, :N, :] = biasT
    bias_dev = np.ascontiguousarray(
        pad.reshape(H, 2, 128, N).transpose(1, 2, 0, 3)
    ).astype(ml_dtypes.bfloat16)
    return wqkvT, wprojT, projb2, bias_dev


def kernel(**inputs):
    x = np.asarray(inputs["x"], dtype=np.float32)
    wqkvT, wprojT, projb2, bias_dev = _prep_host(inputs)

    nc = _get_compiled()
    in_maps = []
    for i in range(N_CORES):
        shard = x[i * BL : (i + 1) * BL].reshape(M, C)
        in_maps.append(
            {
                "xT": np.ascontiguousarray(shard.T).astype(ml_dtypes.bfloat16),
                "wqkvT": wqkvT,
                "wprojT": wprojT,
                "projb": projb2,
                "biasT": bias_dev,
            }
        )
    res = run_bass_kernel_spmd(nc, in_maps, core_ids=list(range(N_CORES)))
    out = np.empty((B, N, C), dtype=np.float32)
    for i in range(N_CORES):
        out[i * BL : (i + 1) * BL] = res.results[i]["out"].reshape(BL, N, C)
    return out


def run_traced(**inputs):
    """Like kernel() but with NTFF tracing; returns (out, BassKernelResults)."""
    x = np.asarray(inputs["x"], dtype=np.float32)
    wqkvT, wprojT, projb2, bias_dev = _prep_host(inputs)
    nc = _get_compiled()
    in_maps = []
    for i in range(N_CORES):
        shard = x[i * BL : (i + 1) * BL].reshape(M, C)
        in_maps.append(
            {
                "xT": np.ascontiguousarray(shard.T).astype(ml_dtypes.bfloat16),
                "wqkvT": wqkvT,
                "wprojT": wprojT,
                "projb": projb2,
                "biasT": bias_dev,
            }
        )
    res = run_bass_kernel_spmd(
        nc, in_maps, core_ids=list(range(N_CORES)), trace=True
    )
    out = np.empty((B, N, C), dtype=np.float32)
    for i in range(N_CORES):
        out[i * BL : (i + 1) * BL] = res.results[i]["out"].reshape(BL, N, C)
    return out, res


# revision 26
# speedup vs baseline: 2.0088x; 1.0445x over previous
"""Trainium2 Bass kernel for a 12-head dense attention block (BEiT-style
windowed attention with relative-position bias), batch-parallel over 8
NeuronCores.

Shapes (hardcoded): x [64, 197, 768], qkv_w [2304, 768], proj_w [768, 768],
proj_b [768], rel_table [732, 12], rel_index [197, 197] int32.

Sharding: data-parallel over batch — each of the 8 cores handles 8 batch
elements end-to-end; no collectives. Host pre-transposes x and the weights
so the device kernel needs no on-chip transposes:

  phase 1: qkT[2C, M] = wqkvT.T-style matmul producing q,k TRANSPOSED
           ([feature, token]) + v in natural layout ([token, feature]),
           bf16 matmuls (fast weight load).
  phase 2: per (batch, head): scoresT[nk, nq] = kT.T @ qT, exp on the
           scalar engine, relative-position bias applied multiplicatively
           (exp(bias) precomputed), softmax denominator via gpsimd
           partition_all_reduce, attention output accumulated TRANSPOSED
           (outT[d, nq] = v.T-free matmul) and normalized by a DVE multiply.
  phase 3: y = attn_outT.T @ projT (bf16) + broadcast bias add,
           DMA out in natural layout.
"""

import sys

if "/opt/trn_rl_repo" not in sys.path:
    sys.path.insert(0, "/opt/trn_rl_repo")

import numpy as np
import ml_dtypes

import concourse.bass as bass  # noqa: F401  (registers rust bindings)
import concourse.tile as tile
from concourse import bacc, bass_isa, mybir
from concourse.bass_utils import run_bass_kernel_spmd

N_CORES = 8
B, N, C, H, D = 64, 197, 768, 12, 64
BL = B // N_CORES            # 8 batch elements per core
M = BL * N                   # 1576 tokens per core
SCALE = D ** -0.5
NK0 = 128
NK1 = N - NK0                # 69
KC = C // 128                # 6 contraction chunks
MT = 4                       # m-tiles in phase 1 (qk part)
MTS = M // MT                # 394
NT_QK = (2 * C) // 128       # 12 output-feature tiles for q,k
MT3 = (M + 127) // 128       # 13 m-tiles in phase 3

F32 = mybir.dt.float32
BF16 = mybir.dt.bfloat16
F32R = mybir.dt.float32r

# self-inverse head<->slot permutation within each 4-head group: consecutive
# score matmuls alternate array row-strips (head parity) and run concurrently,
# so they must target different PSUM banks -> interleave slots (0,2,1,3)
SIG4 = (0, 2, 1, 3)


def sig(h):
    return (h // 4) * 4 + SIG4[h % 4]


_COMPILED = {}


def _build_nc():
    nc = bacc.Bacc(
        "TRN2", target_bir_lowering=False, debug=False, num_devices=N_CORES
    )
    xT = nc.declare_dram_parameter("xT", [C, M], BF16, isOutput=False)
    wqkvT = nc.declare_dram_parameter("wqkvT", [C, 3 * C], BF16, isOutput=False)
    wprojT = nc.declare_dram_parameter("wprojT", [C, C], BF16, isOutput=False)
    projb = nc.declare_dram_parameter("projb", [1, C], F32, isOutput=False)
    biasT = nc.declare_dram_parameter("biasT", [2, 128, H, N], BF16, isOutput=False)
    out_d = nc.declare_dram_parameter("out", [M, C], F32, isOutput=True)

    with tile.TileContext(nc) as tc:
        _body(nc, tc, xT, wqkvT, wprojT, projb, biasT, out_d)
    nc.compile()
    return nc


def _body(nc, tc, xT, wqkvT, wprojT, projb, biasT, out_d):
    exp = mybir.ActivationFunctionType.Exp

    consts = tc.alloc_tile_pool(name="consts", bufs=1)
    ones128 = consts.tile([128, 128], BF16)
    nc.vector.memset(ones128, 1.0)
    projb_sb = consts.tile([1, C], F32)
    nc.sync.dma_start(out=projb_sb[:, :], in_=projb[:, :])
    projb_bc = consts.tile([128, C], F32)
    bias_sb = [consts.tile([128, H, N], BF16, tag=f"bias{t}", name=f"bias{t}") for t in range(2)]
    deferred_dmas = []

    # ---- outputs of phase 1 (persist into phase 2) ----
    qk_pool = tc.alloc_tile_pool(name="qk", bufs=1)
    qkT = [qk_pool.tile([128, M], BF16, tag=f"qk{t}", name=f"qk{t}") for t in range(NT_QK)]
    v_pool = tc.alloc_tile_pool(name="v", bufs=1)
    v_sb = [
        [v_pool.tile([128, C], BF16, tag=f"v{b}_{pt}", name=f"v{b}_{pt}") for pt in range(2)]
        for b in range(BL)
    ]

    ps_mm = tc.alloc_tile_pool(name="psmm", bufs=2, space="PSUM")

    # ---- PE warm-up while the input DMAs land ----
    warm_pool = tc.alloc_tile_pool(name="warm", bufs=1, space="PSUM")
    wtile = warm_pool.tile([128, 512], F32, tag="warm")
    for _ in range(100):
        nc.tensor.matmul(wtile[:, 0:128], ones128[:, :], ones128[:, :],
                         start=True, stop=True)
    warm_pool.release()

    # ---- phase 1: qkT (transposed q,k) and v (natural) ----
    with tc.tile_pool(name="xt", bufs=1) as xt_pool, tc.tile_pool(
        name="wq", bufs=1
    ) as wq_pool:
        xt = [xt_pool.tile([128, M], BF16, tag=f"xt{k}", name=f"xt{k}") for k in range(KC)]
        wq = [wq_pool.tile([128, 3 * C], BF16, tag=f"wq{k}", name=f"wq{k}") for k in range(KC)]
        for k in range(KC):
            nc.sync.dma_start(out=xt[k][:, :], in_=xT[k * 128 : (k + 1) * 128, :])
            last_in_dma = nc.sync.dma_start(
                out=wq[k][:, :], in_=wqkvT[k * 128 : (k + 1) * 128, :]
            )
        # secondary inputs (bias table, proj bias broadcast) wait for the
        # phase-1 inputs so the startup DMA ramp is as short as possible
        _pb = projb[:, :]
        deferred_dmas.append(
            nc.sync.dma_start(
                out=projb_bc[:, :],
                in_=bass.AP(
                    tensor=_pb.tensor, offset=_pb.offset, ap=[[0, 128], [1, C]]
                ),
            )
        )
        for t in range(2):
            deferred_dmas.append(
                nc.sync.dma_start(out=bias_sb[t][:, :, :], in_=biasT[t, :, :, :])
            )
        for d in deferred_dmas:
            tile.add_dep_helper(d.ins, last_in_dma.ins, sync=True, reason="defer-input")

        for mt in range(MT):
            ms = slice(mt * MTS, (mt + 1) * MTS)
            for nt in range(NT_QK):
                ps = ps_mm.tile([128, MTS], F32, tag="ps1")
                for k in range(KC):
                    nc.tensor.matmul(
                        ps[:, :],
                        wq[k][:, nt * 128 : (nt + 1) * 128],
                        xt[k][:, ms],
                        start=(k == 0),
                        stop=(k == KC - 1),
                    )
                nc.any.tensor_copy(qkT[nt][:, ms], ps[:, :])
            for b in (2 * mt, 2 * mt + 1):
                for pt in range(2):
                    psz = NK0 if pt == 0 else NK1
                    mofs = b * N + pt * 128
                    for nt2 in range(2):
                        ps = ps_mm.tile([128, 384], F32, tag="ps1")
                        for k in range(KC):
                            nc.tensor.matmul(
                                ps[:psz, :],
                                xt[k][:, mofs : mofs + psz],
                                wq[k][
                                    :, 2 * C + nt2 * 384 : 2 * C + (nt2 + 1) * 384
                                ],
                                start=(k == 0),
                                stop=(k == KC - 1),
                            )
                        nc.any.tensor_copy(
                            v_sb[b][pt][:psz, nt2 * 384 : (nt2 + 1) * 384],
                            ps[:psz, :],
                        )

    # ---- phase 3 weights: load early into space freed by xt/wq ----
    wp_pool = tc.alloc_tile_pool(name="wp", bufs=1)
    wp = [wp_pool.tile([128, C], BF16, tag=f"wp{k}", name=f"wp{k}") for k in range(KC)]
    for k in range(KC):
        _d = nc.sync.dma_start(
            out=wp[k][:, :], in_=wprojT[k * 128 : (k + 1) * 128, :]
        )
        tile.add_dep_helper(_d.ins, last_in_dma.ins, sync=True, reason="defer-wp")

    ao_pool = tc.alloc_tile_pool(name="ao", bufs=1)
    aoT = [ao_pool.tile([128, M], BF16, tag=f"ao{t}", name=f"ao{t}") for t in range(KC)]

    # ---- phase 2: attention per batch element ----
    et_pool = tc.alloc_tile_pool(name="et", bufs=2)
    raw_pool = tc.alloc_tile_pool(name="raw", bufs=3)
    ar_pool = tc.alloc_tile_pool(name="ar", bufs=2)
    ps_sc = tc.alloc_tile_pool(name="pssc", bufs=2, space="PSUM")
    ps_po = tc.alloc_tile_pool(name="pspo", bufs=2, space="PSUM")

    for b in range(BL):
        et = et_pool.tile([128, 2, H, N], BF16, tag="et")
        ar = ar_pool.tile([128, H, N], F32, tag="ar")
        for hg in range(3):
            pss = []
            for kt in range(2):
                nk = NK0 if kt == 0 else NK1
                kofs = b * N + kt * 128
                ps = ps_sc.tile([128, 4, 256], F32, tag="pssc")
                pss.append(ps)
                for j in range(4):
                    h = hg * 4 + j
                    off = (h % 2) * 64
                    # scoresT[nk, nq] = kT.T @ qT  (scale folded into Wq);
                    # psum slice SIG4[j] so concurrent row-packed MMs use
                    # different banks
                    nc.tensor.matmul(
                        ps[:nk, SIG4[j], 0:N],
                        qkT[6 + h // 2][off : off + 64, kofs : kofs + nk],
                        qkT[h // 2][off : off + 64, b * N : b * N + N],
                        start=True,
                        stop=True,
                    )
                raw = raw_pool.tile([128, 4, N], BF16, tag="raw")
                nc.scalar.activation(raw[:nk, :, :], ps[:nk, :, 0:N], exp)
                # multiplicative relative-position bias: et = exp(z)*exp(bias)
                nc.vector.tensor_mul(
                    et[:nk, kt, hg * 4 : (hg + 1) * 4, :],
                    raw[:nk, :, :],
                    bias_sb[kt][:nk, hg * 4 : (hg + 1) * 4, :],
                )
                # softmax denominator: a ones-row matmul both reduces
                # across partitions and broadcasts the result to all 128
                # output partitions; both nk tiles accumulate into the
                # kt-0 scores psum (reusing it after exp consumed it)
                flat = pss[0].rearrange("p a b -> p (a b)")
                for pr in range(2):
                    nc.tensor.matmul(
                        flat[:, pr * 512 : pr * 512 + 2 * N],
                        ones128[:nk, :],
                        et[:nk, kt, hg * 4 + 2 * pr : hg * 4 + 2 * pr + 2, :],
                        start=(kt == 0),
                        stop=(kt == 1),
                    )
            # reciprocal of the denominators (identical on every partition)
            flat = pss[0].rearrange("p a b -> p (a b)")
            for pr in range(2):
                nc.vector.reciprocal_approx_fast(
                    out=ar[:, hg * 4 + 2 * pr : hg * 4 + 2 * pr + 2, :],
                    in_=flat[:, pr * 512 : pr * 512 + 2 * N],
                )

        for grp in range(3):
            po = ps_po.tile([128, 2, 256], F32, tag="pspo")
            for j in range(4):
                h = grp * 4 + j
                base = (j % 2) * 64
                sl = j // 2
                for kt in range(2):
                    nk = NK0 if kt == 0 else NK1
                    nc.tensor.matmul(
                        po[base : base + 64, sl, 0:N],
                        v_sb[b][kt][:nk, h * 64 : (h + 1) * 64],
                        et[:nk, kt, sig(h), :],
                        start=(kt == 0),
                        stop=(kt == 1),
                        tile_position=(0, base),
                    )
            for j in range(4):
                h = grp * 4 + j
                base = (j % 2) * 64
                sl = j // 2
                nc.vector.tensor_tensor(
                    aoT[h // 2][base : base + 64, b * N : b * N + N],
                    po[base : base + 64, sl, 0:N],
                    ar[base : base + 64, sig(h), :],
                    mybir.AluOpType.mult,
                )

    # ---- phase 3: y = attn_outT.T @ projT + proj_b ----
    with tc.tile_pool(name="ostg", bufs=3) as ostg_pool:
        for mt in range(MT3):
            msz = 128 if mt < MT3 - 1 else M - 128 * (MT3 - 1)
            stg = ostg_pool.tile([128, C], F32, tag="stg")
            for nt2 in range(2):
                ns = slice(nt2 * 384, (nt2 + 1) * 384)
                ps = ps_mm.tile([128, 384], F32, tag="ps1")
                for c in range(KC):
                    nc.tensor.matmul(
                        ps[:msz, :],
                        aoT[c][:, mt * 128 : mt * 128 + msz],
                        wp[c][:, ns],
                        start=(c == 0),
                        stop=(c == KC - 1),
                    )
                nc.any.tensor_add(stg[:msz, ns], ps[:msz, :], projb_bc[:msz, ns])
            nc.sync.dma_start(
                out=out_d[mt * 128 : mt * 128 + msz, :], in_=stg[:msz, :]
            )

    for pool in (
        ps_po,
        ps_sc,
        ar_pool,
        raw_pool,
        et_pool,
        ao_pool,
        wp_pool,
        ps_mm,
        v_pool,
        qk_pool,
        consts,
    ):
        pool.release()


def _get_compiled():
    if "nc" not in _COMPILED:
        _COMPILED["nc"] = _build_nc()
    return _COMPILED["nc"]


def _prep_host(inputs):
    qkv_w = np.asarray(inputs["qkv_w"], dtype=np.float32)
    proj_w = np.asarray(inputs["proj_w"], dtype=np.float32)
    proj_b = np.asarray(inputs["proj_b"], dtype=np.float32)
    rel_table = np.asarray(inputs["rel_table"], dtype=np.float32)
    rel_index = np.asarray(inputs["rel_index"]).astype(np.int64)

    w = qkv_w.copy()
    w[:C] *= SCALE  # fold the attention scale into Wq
    wqkvT = np.ascontiguousarray(w.T).astype(ml_dtypes.bfloat16)
    wprojT = np.ascontiguousarray(proj_w.T).astype(ml_dtypes.bfloat16)
    projb2 = np.ascontiguousarray(proj_b.reshape(1, C))

    bias_full = rel_table[rel_index]          # [nq, nk, H]
    biasT = np.exp(bias_full.transpose(2, 1, 0))  # [H, nk, nq], exp for the
    # multiplicative-bias trick: exp(z + b) = exp(z) * exp(b)
    perm = [(t // 4) * 4 + (0, 2, 1, 3)[t % 4] for t in range(H)]
    biasT = biasT[perm]
    pad = np.zeros((H, 2 * 128, N), np.float32)
    pad[:, :N, :] = biasT
    bias_dev = np.ascontiguousarray(
        pad.reshape(H, 2, 128, N).transpose(1, 2, 0, 3)
    ).astype(ml_dtypes.bfloat16)
    return wqkvT, wprojT, projb2, bias_dev


def kernel(**inputs):
    x = np.asarray(inputs["x"], dtype=np.float32)
    wqkvT, wprojT, projb2, bias_dev = _prep_host(inputs)

    nc = _get_compiled()
    in_maps = []
    for i in range(N_CORES):
        shard = x[i * BL : (i + 1) * BL].reshape(M, C)
        in_maps.append(
            {
                "xT": np.ascontiguousarray(shard.T).astype(ml_dtypes.bfloat16),
                "wqkvT": wqkvT,
                "wprojT": wprojT,
                "projb": projb2,
                "biasT": bias_dev,
            }
        )
    res = run_bass_kernel_spmd(nc, in_maps, core_ids=list(range(N_CORES)))
    out = np.empty((B, N, C), dtype=np.float32)
    for i in range(N_CORES):
        out[i * BL : (i + 1) * BL] = res.results[i]["out"].reshape(BL, N, C)
    return out


def run_traced(**inputs):
    """Like kernel() but with NTFF tracing; returns (out, BassKernelResults)."""
    x = np.asarray(inputs["x"], dtype=np.float32)
    wqkvT, wprojT, projb2, bias_dev = _prep_host(inputs)
    nc = _get_compiled()
    in_maps = []
    for i in range(N_CORES):
        shard = x[i * BL : (i + 1) * BL].reshape(M, C)
        in_maps.append(
            {
                "xT": np.ascontiguousarray(shard.T).astype(ml_dtypes.bfloat16),
                "wqkvT": wqkvT,
                "wprojT": wprojT,
                "projb": projb2,
                "biasT": bias_dev,
            }
        )
    res = run_bass_kernel_spmd(
        nc, in_maps, core_ids=list(range(N_CORES)), trace=True
    )
    out = np.empty((B, N, C), dtype=np.float32)
    for i in range(N_CORES):
        out[i * BL : (i + 1) * BL] = res.results[i]["out"].reshape(BL, N, C)
    return out, res


# revision 27
# speedup vs baseline: 2.0487x; 1.0199x over previous
"""Trainium2 Bass kernel for a 12-head dense attention block (BEiT-style
windowed attention with relative-position bias), batch-parallel over 8
NeuronCores.

Shapes (hardcoded): x [64, 197, 768], qkv_w [2304, 768], proj_w [768, 768],
proj_b [768], rel_table [732, 12], rel_index [197, 197] int32.

Sharding: data-parallel over batch — each of the 8 cores handles 8 batch
elements end-to-end; no collectives. Host pre-transposes x and the weights
so the device kernel needs no on-chip transposes:

  phase 1: qkT[2C, M] = wqkvT.T-style matmul producing q,k TRANSPOSED
           ([feature, token]) + v in natural layout ([token, feature]),
           bf16 matmuls (fast weight load).
  phase 2: per (batch, head): scoresT[nk, nq] = kT.T @ qT, exp on the
           scalar engine, relative-position bias applied multiplicatively
           (exp(bias) precomputed), softmax denominator via gpsimd
           partition_all_reduce, attention output accumulated TRANSPOSED
           (outT[d, nq] = v.T-free matmul) and normalized by a DVE multiply.
  phase 3: y = attn_outT.T @ projT (bf16) + broadcast bias add,
           DMA out in natural layout.
"""

import sys

if "/opt/trn_rl_repo" not in sys.path:
    sys.path.insert(0, "/opt/trn_rl_repo")

import numpy as np
import ml_dtypes

import concourse.bass as bass  # noqa: F401  (registers rust bindings)
import concourse.tile as tile
from concourse import bacc, bass_isa, mybir
from concourse.bass_utils import run_bass_kernel_spmd

N_CORES = 8
B, N, C, H, D = 64, 197, 768, 12, 64
BL = B // N_CORES            # 8 batch elements per core
M = BL * N                   # 1576 tokens per core
SCALE = D ** -0.5
NK0 = 128
NK1 = N - NK0                # 69
KC = C // 128                # 6 contraction chunks
MT = 4                       # m-tiles in phase 1 (qk part)
MTS = M // MT                # 394
NT_QK = (2 * C) // 128       # 12 output-feature tiles for q,k
MT3 = (M + 127) // 128       # 13 m-tiles in phase 3

F32 = mybir.dt.float32
BF16 = mybir.dt.bfloat16
F32R = mybir.dt.float32r

# self-inverse head<->slot permutation within each 4-head group: consecutive
# score matmuls alternate array row-strips (head parity) and run concurrently,
# so they must target different PSUM banks -> interleave slots (0,2,1,3)
SIG4 = (0, 2, 1, 3)


def sig(h):
    return (h // 4) * 4 + SIG4[h % 4]


_COMPILED = {}


def _build_nc():
    nc = bacc.Bacc(
        "TRN2", target_bir_lowering=False, debug=False, num_devices=N_CORES
    )
    xT = nc.declare_dram_parameter("xT", [C, M], BF16, isOutput=False)
    wqkvT = nc.declare_dram_parameter("wqkvT", [C, 3 * C], BF16, isOutput=False)
    wprojT = nc.declare_dram_parameter("wprojT", [C, C], BF16, isOutput=False)
    projb = nc.declare_dram_parameter("projb", [1, C], F32, isOutput=False)
    biasT = nc.declare_dram_parameter("biasT", [2, 128, H, N], BF16, isOutput=False)
    out_d = nc.declare_dram_parameter("out", [M, C], F32, isOutput=True)

    with tile.TileContext(nc) as tc:
        _body(nc, tc, xT, wqkvT, wprojT, projb, biasT, out_d)
    nc.compile()
    return nc


def _body(nc, tc, xT, wqkvT, wprojT, projb, biasT, out_d):
    exp = mybir.ActivationFunctionType.Exp

    consts = tc.alloc_tile_pool(name="consts", bufs=1)
    ones128 = consts.tile([128, 128], BF16)
    nc.vector.memset(ones128, 1.0)
    projb_sb = consts.tile([1, C], F32)
    nc.sync.dma_start(out=projb_sb[:, :], in_=projb[:, :])
    projb_bc = consts.tile([128, C], F32)
    bias_sb = [consts.tile([128, H, N], BF16, tag=f"bias{t}", name=f"bias{t}") for t in range(2)]
    deferred_dmas = []

    # ---- outputs of phase 1 (persist into phase 2) ----
    qk_pool = tc.alloc_tile_pool(name="qk", bufs=1)
    qkT = [qk_pool.tile([128, M], BF16, tag=f"qk{t}", name=f"qk{t}") for t in range(NT_QK)]
    v_pool = tc.alloc_tile_pool(name="v", bufs=1)
    v_sb = [
        [v_pool.tile([128, C], BF16, tag=f"v{b}_{pt}", name=f"v{b}_{pt}") for pt in range(2)]
        for b in range(BL)
    ]

    ps_mm = tc.alloc_tile_pool(name="psmm", bufs=2, space="PSUM")

    # ---- PE warm-up while the input DMAs land ----
    warm_pool = tc.alloc_tile_pool(name="warm", bufs=1, space="PSUM")
    wtile = warm_pool.tile([128, 512], F32, tag="warm")
    for _ in range(100):
        nc.tensor.matmul(wtile[:, 0:128], ones128[:, :], ones128[:, :],
                         start=True, stop=True)
    warm_pool.release()

    # ---- phase 1: qkT (transposed q,k) and v (natural) ----
    with tc.tile_pool(name="xt", bufs=1) as xt_pool, tc.tile_pool(
        name="wq", bufs=1
    ) as wq_pool:
        xt = [xt_pool.tile([128, M], BF16, tag=f"xt{k}", name=f"xt{k}") for k in range(KC)]
        wq = [wq_pool.tile([128, 3 * C], BF16, tag=f"wq{k}", name=f"wq{k}") for k in range(KC)]
        for k in range(KC):
            nc.sync.dma_start(out=xt[k][:, :], in_=xT[k * 128 : (k + 1) * 128, :])
            last_in_dma = nc.sync.dma_start(
                out=wq[k][:, :], in_=wqkvT[k * 128 : (k + 1) * 128, :]
            )
        # secondary inputs (bias table, proj bias broadcast) wait for the
        # phase-1 inputs so the startup DMA ramp is as short as possible
        _pb = projb[:, :]
        deferred_dmas.append(
            nc.sync.dma_start(
                out=projb_bc[:, :],
                in_=bass.AP(
                    tensor=_pb.tensor, offset=_pb.offset, ap=[[0, 128], [1, C]]
                ),
            )
        )
        for t in range(2):
            deferred_dmas.append(
                nc.sync.dma_start(out=bias_sb[t][:, :, :], in_=biasT[t, :, :, :])
            )
        for d in deferred_dmas:
            tile.add_dep_helper(d.ins, last_in_dma.ins, sync=True, reason="defer-input")

        for mt in range(MT):
            ms = slice(mt * MTS, (mt + 1) * MTS)
            for nt in range(NT_QK):
                ps = ps_mm.tile([128, MTS], F32, tag="ps1")
                for k in range(KC):
                    nc.tensor.matmul(
                        ps[:, :],
                        wq[k][:, nt * 128 : (nt + 1) * 128],
                        xt[k][:, ms],
                        start=(k == 0),
                        stop=(k == KC - 1),
                    )
                nc.any.tensor_copy(qkT[nt][:, ms], ps[:, :])
            for b in (2 * mt, 2 * mt + 1):
                for pt in range(2):
                    psz = NK0 if pt == 0 else NK1
                    mofs = b * N + pt * 128
                    for nt2 in range(2):
                        ps = ps_mm.tile([128, 384], F32, tag="ps1")
                        for k in range(KC):
                            nc.tensor.matmul(
                                ps[:psz, :],
                                xt[k][:, mofs : mofs + psz],
                                wq[k][
                                    :, 2 * C + nt2 * 384 : 2 * C + (nt2 + 1) * 384
                                ],
                                start=(k == 0),
                                stop=(k == KC - 1),
                            )
                        nc.any.tensor_copy(
                            v_sb[b][pt][:psz, nt2 * 384 : (nt2 + 1) * 384],
                            ps[:psz, :],
                        )

    # ---- phase 3 weights: load early into space freed by xt/wq ----
    wp_pool = tc.alloc_tile_pool(name="wp", bufs=1)
    wp = [wp_pool.tile([128, C], BF16, tag=f"wp{k}", name=f"wp{k}") for k in range(KC)]
    for k in range(KC):
        _d = nc.sync.dma_start(
            out=wp[k][:, :], in_=wprojT[k * 128 : (k + 1) * 128, :]
        )
        tile.add_dep_helper(_d.ins, last_in_dma.ins, sync=True, reason="defer-wp")

    ao_pool = tc.alloc_tile_pool(name="ao", bufs=1)
    aoT = [ao_pool.tile([128, M], BF16, tag=f"ao{t}", name=f"ao{t}") for t in range(KC)]

    # ---- phase 2: attention per batch element ----
    et_pool = tc.alloc_tile_pool(name="et", bufs=3)
    raw_pool = tc.alloc_tile_pool(name="raw", bufs=4)
    ar_pool = tc.alloc_tile_pool(name="ar", bufs=2)
    ps_sc = tc.alloc_tile_pool(name="pssc", bufs=2, space="PSUM")
    ps_po = tc.alloc_tile_pool(name="pspo", bufs=2, space="PSUM")

    for b in range(BL):
        et = et_pool.tile([128, 2, H, N], BF16, tag="et")
        ar = ar_pool.tile([128, H, N], F32, tag="ar")
        for hg in range(3):
            pss = []
            for kt in range(2):
                nk = NK0 if kt == 0 else NK1
                kofs = b * N + kt * 128
                ps = ps_sc.tile([128, 4, 256], F32, tag="pssc")
                pss.append(ps)
                for j in range(4):
                    h = hg * 4 + j
                    off = (h % 2) * 64
                    # scoresT[nk, nq] = kT.T @ qT  (scale folded into Wq);
                    # psum slice SIG4[j] so concurrent row-packed MMs use
                    # different banks
                    nc.tensor.matmul(
                        ps[:nk, SIG4[j], 0:N],
                        qkT[6 + h // 2][off : off + 64, kofs : kofs + nk],
                        qkT[h // 2][off : off + 64, b * N : b * N + N],
                        start=True,
                        stop=True,
                    )
                raw = raw_pool.tile([128, 4, N], BF16, tag="raw")
                nc.scalar.activation(raw[:nk, :, :], ps[:nk, :, 0:N], exp)
                # multiplicative relative-position bias: et = exp(z)*exp(bias)
                nc.vector.tensor_mul(
                    et[:nk, kt, hg * 4 : (hg + 1) * 4, :],
                    raw[:nk, :, :],
                    bias_sb[kt][:nk, hg * 4 : (hg + 1) * 4, :],
                )
                # softmax denominator: a ones-row matmul both reduces
                # across partitions and broadcasts the result to all 128
                # output partitions; both nk tiles accumulate into the
                # kt-0 scores psum (reusing it after exp consumed it)
                flat = pss[0].rearrange("p a b -> p (a b)")
                for pr in range(2):
                    nc.tensor.matmul(
                        flat[:, pr * 512 : pr * 512 + 2 * N],
                        ones128[:nk, :],
                        et[:nk, kt, hg * 4 + 2 * pr : hg * 4 + 2 * pr + 2, :],
                        start=(kt == 0),
                        stop=(kt == 1),
                    )
            # reciprocal of the denominators (identical on every partition)
            flat = pss[0].rearrange("p a b -> p (a b)")
            for pr in range(2):
                nc.vector.reciprocal_approx_fast(
                    out=ar[:, hg * 4 + 2 * pr : hg * 4 + 2 * pr + 2, :],
                    in_=flat[:, pr * 512 : pr * 512 + 2 * N],
                )

        for grp in range(3):
            po = ps_po.tile([128, 2, 256], F32, tag="pspo")
            for j in range(4):
                h = grp * 4 + j
                base = (j % 2) * 64
                sl = j // 2
                for kt in range(2):
                    nk = NK0 if kt == 0 else NK1
                    nc.tensor.matmul(
                        po[base : base + 64, sl, 0:N],
                        v_sb[b][kt][:nk, h * 64 : (h + 1) * 64],
                        et[:nk, kt, sig(h), :],
                        start=(kt == 0),
                        stop=(kt == 1),
                        tile_position=(0, base),
                    )
            for j in range(4):
                h = grp * 4 + j
                base = (j % 2) * 64
                sl = j // 2
                nc.vector.tensor_tensor(
                    aoT[h // 2][base : base + 64, b * N : b * N + N],
                    po[base : base + 64, sl, 0:N],
                    ar[base : base + 64, sig(h), :],
                    mybir.AluOpType.mult,
                )

    # ---- phase 3: y = attn_outT.T @ projT + proj_b ----
    with tc.tile_pool(name="ostg", bufs=3) as ostg_pool:
        for mt in range(MT3):
            msz = 128 if mt < MT3 - 1 else M - 128 * (MT3 - 1)
            stg = ostg_pool.tile([128, C], F32, tag="stg")
            for nt2 in range(2):
                ns = slice(nt2 * 384, (nt2 + 1) * 384)
                ps = ps_mm.tile([128, 384], F32, tag="ps1")
                for c in range(KC):
                    nc.tensor.matmul(
                        ps[:msz, :],
                        aoT[c][:, mt * 128 : mt * 128 + msz],
                        wp[c][:, ns],
                        start=(c == 0),
                        stop=(c == KC - 1),
                    )
                nc.any.tensor_add(stg[:msz, ns], ps[:msz, :], projb_bc[:msz, ns])
            nc.sync.dma_start(
                out=out_d[mt * 128 : mt * 128 + msz, :], in_=stg[:msz, :]
            )

    for pool in (
        ps_po,
        ps_sc,
        ar_pool,
        raw_pool,
        et_pool,
        ao_pool,
        wp_pool,
        ps_mm,
        v_pool,
        qk_pool,
        consts,
    ):
        pool.release()


def _get_compiled():
    if "nc" not in _COMPILED:
        _COMPILED["nc"] = _build_nc()
    return _COMPILED["nc"]


def _prep_host(inputs):
    qkv_w = np.asarray(inputs["qkv_w"], dtype=np.float32)
    proj_w = np.asarray(inputs["proj_w"], dtype=np.float32)
    proj_b = np.asarray(inputs["proj_b"], dtype=np.float32)
    rel_table = np.asarray(inputs["rel_table"], dtype=np.float32)
    rel_index = np.asarray(inputs["rel_index"]).astype(np.int64)

    w = qkv_w.copy()
    w[:C] *= SCALE  # fold the attention scale into Wq
    wqkvT = np.ascontiguousarray(w.T).astype(ml_dtypes.bfloat16)
    wprojT = np.ascontiguousarray(proj_w.T).astype(ml_dtypes.bfloat16)
    projb2 = np.ascontiguousarray(proj_b.reshape(1, C))

    bias_full = rel_table[rel_index]          # [nq, nk, H]
    biasT = np.exp(bias_full.transpose(2, 1, 0))  # [H, nk, nq], exp for the
    # multiplicative-bias trick: exp(z + b) = exp(z) * exp(b)
    perm = [(t // 4) * 4 + (0, 2, 1, 3)[t % 4] for t in range(H)]
    biasT = biasT[perm]
    pad = np.zeros((H, 2 * 128, N), np.float32)
    pad[:, :N, :] = biasT
    bias_dev = np.ascontiguousarray(
        pad.reshape(H, 2, 128, N).transpose(1, 2, 0, 3)
    ).astype(ml_dtypes.bfloat16)
    return wqkvT, wprojT, projb2, bias_dev


def kernel(**inputs):
    x = np.asarray(inputs["x"], dtype=np.float32)
    wqkvT, wprojT, projb2, bias_dev = _prep_host(inputs)

    nc = _get_compiled()
    in_maps = []
    for i in range(N_CORES):
        shard = x[i * BL : (i + 1) * BL].reshape(M, C)
        in_maps.append(
            {
                "xT": np.ascontiguousarray(shard.T).astype(ml_dtypes.bfloat16),
                "wqkvT": wqkvT,
                "wprojT": wprojT,
                "projb": projb2,
                "biasT": bias_dev,
            }
        )
    res = run_bass_kernel_spmd(nc, in_maps, core_ids=list(range(N_CORES)))
    out = np.empty((B, N, C), dtype=np.float32)
    for i in range(N_CORES):
        out[i * BL : (i + 1) * BL] = res.results[i]["out"].reshape(BL, N, C)
    return out


def run_traced(**inputs):
    """Like kernel() but with NTFF tracing; returns (out, BassKernelResults)."""
    x = np.asarray(inputs["x"], dtype=np.float32)
    wqkvT, wprojT, projb2, bias_dev = _prep_host(inputs)
    nc = _get_compiled()
    in_maps = []
    for i in range(N_CORES):
        shard = x[i * BL : (i + 1) * BL].reshape(M, C)
        in_maps.append(
            {
                "xT": np.ascontiguousarray(shard.T).astype(ml_dtypes.bfloat16),
                "wqkvT": wqkvT,
                "wprojT": wprojT,
                "projb": projb2,
                "biasT": bias_dev,
            }
        )
    res = run_bass_kernel_spmd(
        nc, in_maps, core_ids=list(range(N_CORES)), trace=True
    )
    out = np.empty((B, N, C), dtype=np.float32)
    for i in range(N_CORES):
        out[i * BL : (i + 1) * BL] = res.results[i]["out"].reshape(BL, N, C)
    return out, res
